# revision 5
# baseline (speedup 1.0000x reference)
"""GCN (2-layer, symmetric-normalized adjacency) on 8 TRN2 NeuronCores.

Strategy:
  - Full f32. Host does graph preprocessing only (normalization constants,
    partitioning, index tables); all FLOPs on x/W run on device.
  - Nodes padded to NP; dst tiles of 128 nodes; tiles assigned to cores with
    count balancing (sort by edge count, deal round-robin).
  - GEMM1 per-core computes h' = dinv[src]*(x@W1) for ALL nodes into a DRAM
    table (per-core column order: own tiles first, so self-loop rows sit at
    static offsets).
  - Edge messages gathered from the table with bulk dma_gather (int16
    indices -> 4 src-row chunks; per-(tile,chunk) quotas padded to the max
    across cores so the program is SPMD-uniform).
  - Aggregation = one-hot matmuls on the TensorEngine: S[p,j] =
    (dstloc[p]==j), PSUM-accumulated per dst tile; self-loop row added from
    the table; epilogue relu(dinv*(agg+own)+b1).
  - z' = dinv*(h1@W2) shards AllGathered into a z table; layer 2 repeats the
    gather/aggregate structure and ends in log_softmax.
"""

import sys
import types

import numpy as np


def _install_ntff_hook():
    if "antenv.axon_hooks" in sys.modules:
        return
    try:
        from trn_agent_boot.trn_boot import _ntff_profile_via_ctypes

        hook = _ntff_profile_via_ctypes("/opt/axon/libaxon_pjrt.so")
    except Exception:
        hook = None
    mod = types.ModuleType("antenv.axon_hooks")
    mod.get_axon_ntff_profile_hook = lambda: hook
    mod.set_axon_ntff_profile_hook = lambda h: None
    sys.modules["antenv.axon_hooks"] = mod


_install_ntff_hook()

import concourse.bass as bass
import concourse.bacc as bacc
import concourse.tile as tile
import concourse.mybir as mybir
from concourse.bass_utils import run_bass_kernel_spmd
from concourse.masks import make_identity

P = 128
NCORES = 8
NCHUNK = 4
G = 4        # dst tiles per superbatch
SG = 16      # one-hot columns per S tile
XB = 8       # GEMM1 tiles per staging/DMA batch
INVALID = 1000.0


def _round_up(x, m):
    return -(-x // m) * m


def _build_layer_schedule(lists, q16, supers, TPC):
    """Static (core-uniform) schedule for one layer + per-core dstloc/idx.

    lists[c][s][ch] = (rows_in_chunk, dst_local) per core/slot/chunk,
    q16[s, ch] = padded (max-over-cores) count, multiple of 16.
    """
    meta = {"supers": []}
    dl_cols = []     # column fill instructions: (col, seg info)
    col_count = 0
    idx16_count = 0
    idxw = [[] for _ in range(NCORES)]

    for slots in supers:
        calls = []
        segs = []    # (slot, start_pos, qlen, chunk)
        pos = 0
        for ch in range(NCHUNK):
            q_sum = int(sum(q16[s, ch] for s in slots))
            if q_sum == 0:
                continue
            Q = _round_up(q_sum, P)
            calls.append({"chunk": ch, "Q": Q, "i16": idx16_count,
                          "blk0": pos // P})
            for c in range(NCORES):
                arr = np.zeros(Q, np.int64)
                o = 0
                for s in slots:
                    q = int(q16[s, ch])
                    if q == 0:
                        continue
                    rows = lists[c][s][ch][0]
                    arr[o : o + len(rows)] = rows
                    o += q
                w = arr.astype(np.int16).reshape(Q // 16, 16).T
                idxw[c].append(np.tile(w, (NCORES, 1)))
            for s in slots:
                q = int(q16[s, ch])
                if q:
                    segs.append((s, pos, q, ch))
                    pos += q
            pos += Q - q_sum
            idx16_count += Q // 16
        M = pos
        assert M % P == 0
        nb = M // P
        sides = []
        slot_sides = {}
        for b in range(nb):
            lo, hi = b * P, (b + 1) * P
            for (s, sp, ln, ch) in segs:
                if sp < hi and sp + ln > lo:
                    sd = {"b": b, "col": col_count, "slot": s,
                          "sp": sp, "ln": ln, "chunk": ch}
                    sides.append(sd)
                    slot_sides.setdefault(s, []).append(sd)
                    col_count += 1
        for s, sl in slot_sides.items():
            for sd in sl:
                sd["start"] = sd is sl[0]
                sd["stop"] = sd is sl[-1]
        meta["supers"].append({"slots": slots, "calls": calls, "nb": nb,
                               "sides": sides})
    meta["ncols"] = max(col_count, 1)
    meta["nidx16"] = max(idx16_count, 1)

    dl = [np.full((P, meta["ncols"]), INVALID, np.float32)
          for _ in range(NCORES)]
    for sup in meta["supers"]:
        for sd in sup["sides"]:
            sp, ln, s, ch = sd["sp"], sd["ln"], sd["slot"], sd["chunk"]
            lo, hi = sd["b"] * P, (sd["b"] + 1) * P
            a = max(sp, lo)
            b_ = min(sp + ln, hi)
            r0 = a - sp
            for c in range(NCORES):
                ed = lists[c][s][ch][1]
                r1 = min(b_ - sp, len(ed))
                if r1 > r0:
                    pidx = (a - lo) + np.arange(r1 - r0)
                    dl[c][pidx, sd["col"]] = ed[r0:r1]

    idxcat = []
    for c in range(NCORES):
        if idxw[c]:
            idxcat.append(np.ascontiguousarray(
                np.concatenate(idxw[c], axis=1)))
        else:
            idxcat.append(np.zeros((P, 1), np.int16))
    return meta, dl, idxcat


def _plan(x, W1, b1, W2, b2, edge_index):
    N, F_IN = x.shape
    H = W1.shape[1]
    C = W2.shape[1]
    src = np.asarray(edge_index[0], dtype=np.int64)
    dst = np.asarray(edge_index[1], dtype=np.int64)

    NP = _round_up(N, P * NCORES)
    T = NP // P
    TPC = T // NCORES
    CH = NP // NCHUNK
    assert CH <= 32767 and CH % P == 0

    deg = np.bincount(dst, minlength=NP).astype(np.float64) + 1.0
    dinv = (1.0 / np.sqrt(deg)).astype(np.float32)
    dinv[N:] = 0.0

    etile = dst // P
    counts = np.bincount(etile, minlength=T)
    order = np.argsort(-counts, kind="stable")
    tile_at = order.reshape(TPC, NCORES)            # [slot, core] -> tile
    tile_core = np.empty(T, np.int64)
    tile_slot = np.empty(T, np.int64)
    tile_core[tile_at.reshape(-1)] = np.tile(np.arange(NCORES), TPC)
    tile_slot[tile_at.reshape(-1)] = np.repeat(np.arange(TPC), NCORES)

    nodes = np.arange(NP, dtype=np.int64)
    zrow = tile_core[nodes // P] * TPC * P + tile_slot[nodes // P] * P + nodes % P

    hrow = []
    colnode = []
    for c in range(NCORES):
        own = list(tile_at[:, c])
        rest = [t for t in range(T) if tile_core[t] != c]
        sq = np.array(own + rest, dtype=np.int64)
        pos = np.empty(T, np.int64)
        pos[sq] = np.arange(T)
        hrow.append(pos[nodes // P] * P + nodes % P)
        colnode.append((sq[:, None] * P + np.arange(P)[None, :]).reshape(-1))

    ecore = tile_core[etile]
    eslot = tile_slot[etile]
    edloc = dst % P
    supers = [list(range(g, min(g + G, TPC))) for g in range(0, TPC, G)]

    def edge_lists(erow_per_core):
        lists = [[[None] * NCHUNK for _ in range(TPC)] for _ in range(NCORES)]
        for c in range(NCORES):
            m = ecore == c
            er = erow_per_core[c][m]
            es = eslot[m]
            ed = edloc[m]
            ech = er // CH
            key = np.lexsort((ed, ech, es))
            er, es, ed, ech = er[key], es[key], ed[key], ech[key]
            grp = es * NCHUNK + ech
            bounds = np.searchsorted(grp, np.arange(TPC * NCHUNK + 1))
            for s in range(TPC):
                for ch in range(NCHUNK):
                    a, b = bounds[s * NCHUNK + ch], bounds[s * NCHUNK + ch + 1]
                    lists[c][s][ch] = (er[a:b] - ch * CH, ed[a:b])
        q16 = np.zeros((TPC, NCHUNK), np.int64)
        for s in range(TPC):
            for ch in range(NCHUNK):
                mx = max(len(lists[c][s][ch][0]) for c in range(NCORES))
                q16[s, ch] = _round_up(mx, 16) if mx else 0
        return lists, q16

    l1, q1 = edge_lists([hrow[c][src] for c in range(NCORES)])
    meta1, dl1, ix1 = _build_layer_schedule(l1, q1, supers, TPC)
    zr = zrow[src]
    l2, q2 = edge_lists([zr for _ in range(NCORES)])
    meta2, dl2, ix2 = _build_layer_schedule(l2, q2, supers, TPC)

    plan = {
        "N": N, "F_IN": F_IN, "H": H, "C": C, "NP": NP, "T": T, "TPC": TPC,
        "CH": CH, "meta1": meta1, "meta2": meta2, "tile_at": tile_at,
    }
    xpad = np.zeros((NP, F_IN), np.float32)
    xpad[:N] = np.asarray(x, np.float32)
    percore = []
    for c in range(NCORES):
        xT = np.ascontiguousarray(xpad[colnode[c]].T)
        dinvsrc = np.ascontiguousarray(dinv[colnode[c]].reshape(T, P).T)
        ownnodes = (tile_at[:, c][:, None] * P +
                    np.arange(P)[None, :]).reshape(-1)
        dinvdst = np.ascontiguousarray(dinv[ownnodes].reshape(TPC, P).T)
        percore.append({
            "xT": xT,
            "W1": np.asarray(W1, np.float32),
            "W2": np.asarray(W2, np.float32),
            "b1b": np.tile(np.asarray(b1, np.float32)[None, :], (P, 1)),
            "b2b": np.tile(np.asarray(b2, np.float32)[None, :], (P, 1)),
            "iota": np.tile(np.arange(P, dtype=np.float32)[None, :], (P, 1)),
            "dinvsrc": dinvsrc,
            "dinvdst": dinvdst,
            "dl1": dl1[c], "ix1": ix1[c],
            "dl2": dl2[c], "ix2": ix2[c],
        })
    plan["percore"] = percore
    return plan


def _build(plan):
    F_IN, H, C = plan["F_IN"], plan["H"], plan["C"]
    NP, T, TPC, CH = plan["NP"], plan["T"], plan["TPC"], plan["CH"]
    pc0 = plan["percore"][0]
    f32 = mybir.dt.float32

    nc = bacc.Bacc("TRN2", target_bir_lowering=False, debug=False,
                   num_devices=NCORES)
    t_xT = nc.dram_tensor("xT", [F_IN, NP], f32, kind="ExternalInput")
    t_W1 = nc.dram_tensor("W1", [F_IN, H], f32, kind="ExternalInput")
    t_W2 = nc.dram_tensor("W2", [H, C], f32, kind="ExternalInput")
    t_b1b = nc.dram_tensor("b1b", [P, H], f32, kind="ExternalInput")
    t_b2b = nc.dram_tensor("b2b", [P, C], f32, kind="ExternalInput")
    t_iota = nc.dram_tensor("iota", [P, P], f32, kind="ExternalInput")
    t_dsrc = nc.dram_tensor("dinvsrc", [P, T], f32, kind="ExternalInput")
    t_ddst = nc.dram_tensor("dinvdst", [P, TPC], f32, kind="ExternalInput")
    t_dl1 = nc.dram_tensor("dl1", list(pc0["dl1"].shape), f32,
                           kind="ExternalInput")
    t_ix1 = nc.dram_tensor("ix1", list(pc0["ix1"].shape), mybir.dt.int16,
                           kind="ExternalInput")
    t_dl2 = nc.dram_tensor("dl2", list(pc0["dl2"].shape), f32,
                           kind="ExternalInput")
    t_ix2 = nc.dram_tensor("ix2", list(pc0["ix2"].shape), mybir.dt.int16,
                           kind="ExternalInput")
    t_emb = nc.dram_tensor("emb", [TPC * P, H], f32, kind="ExternalOutput")
    t_logp = nc.dram_tensor("logp", [TPC * P, C], f32, kind="ExternalOutput")

    with tile.TileContext(nc) as tc:
        with (
            tc.tile_pool(name="const", bufs=1) as cp,
            tc.tile_pool(name="xin", bufs=2) as xp,
            tc.tile_pool(name="hst", bufs=3) as hp,
            tc.tile_pool(name="gbuf", bufs=2) as gp,
            tc.tile_pool(name="spool", bufs=3) as sp,
            tc.tile_pool(name="epool", bufs=4) as ep,
            tc.tile_pool(name="opool", bufs=3) as op,
            tc.tile_pool(name="psagg", bufs=G, space="PSUM") as pp,
            tc.tile_pool(name="dram", bufs=1, space="DRAM") as dp,
        ):
            def load_const(t, shape, nm, dtype=f32):
                s = cp.tile(shape, dtype, name=nm, tag=nm)
                nc.sync.dma_start(out=s[:], in_=t[:, :])
                return s

            W1s = load_const(t_W1, [F_IN, H], "cW1")
            W2s = load_const(t_W2, [H, C], "cW2")
            b1s = load_const(t_b1b, [P, H], "cb1")
            b2s = load_const(t_b2b, [P, C], "cb2")
            iot = load_const(t_iota, [P, P], "ciota")
            dsrc = load_const(t_dsrc, [P, T], "cdsrc")
            ddst = load_const(t_ddst, [P, TPC], "cddst")
            dl1s = load_const(t_dl1, list(pc0["dl1"].shape), "cdl1")
            ix1s = load_const(t_ix1, list(pc0["ix1"].shape), "cix1", mybir.dt.int16)
            dl2s = load_const(t_dl2, list(pc0["dl2"].shape), "cdl2")
            ix2s = load_const(t_ix2, list(pc0["ix2"].shape), "cix2", mybir.dt.int16)
            ident = cp.tile([P, P], f32)
            make_identity(nc, ident[:])

            htab = dp.tile([NP, H], f32)
            zb = dp.tile([TPC * P, C], f32)
            ztab = dp.tile([NP, C], f32)

            # ---- Phase A: h' table
            XCW = 28 * P if NP % (28 * P) == 0 else P
            ntc = XCW // P
            for x0 in range(0, NP, XCW):
                xc = xp.tile([F_IN, XCW], f32, tag="xc")
                nc.sync.dma_start(out=xc[:], in_=t_xT[:, x0 : x0 + XCW])
                for jj in range(0, ntc, XB):
                    nbh = min(XB, ntc - jj)
                    hs = hp.tile([P, XB * H], f32, tag="hs")
                    for k in range(nbh):
                        gt = (x0 // P) + jj + k
                        psA = pp.tile([P, H], f32, tag="agg")
                        nc.tensor.matmul(
                            out=psA[:],
                            lhsT=xc[:, (jj + k) * P : (jj + k + 1) * P],
                            rhs=W1s[:],
                            start=True, stop=True,
                        )
                        nc.vector.tensor_tensor(
                            out=hs[:, k * H : (k + 1) * H], in0=psA[:],
                            in1=dsrc[:, gt : gt + 1].to_broadcast([P, H]),
                            op=mybir.AluOpType.mult,
                        )
                    r0 = (x0 // P + jj) * P
                    nc.sync.dma_start(
                        out=htab[r0 : r0 + nbh * P, :].rearrange(
                            "(a p) h -> p a h", p=P),
                        in_=hs[:, : nbh * H].rearrange("p (a h) -> p a h", h=H),
                    )

            def agg_layer(meta, dls, ixs, table, Fdim, epilogue):
                for sup in meta["supers"]:
                    nb = sup["nb"]
                    psums = {}
                    if nb:
                        gb = gp.tile([P, nb * Fdim], f32, tag="gb")
                        for call in sup["calls"]:
                            ch, Q = call["chunk"], call["Q"]
                            d0 = call["blk0"] * Fdim
                            nc.gpsimd.dma_gather(
                                gb[:, d0 : d0 + (Q // P) * Fdim].rearrange(
                                    "p (a h) -> p a h", h=Fdim),
                                table[ch * CH : (ch + 1) * CH, :],
                                ixs[:, call["i16"] : call["i16"] + Q // 16],
                                Q, Q, Fdim,
                                single_packet=False,
                            )
                        sides = sup["sides"]
                        for i0 in range(0, len(sides), SG):
                            grp = sides[i0 : i0 + SG]
                            k = len(grp)
                            st = sp.tile([P, SG * P], f32, tag="s")
                            c0 = grp[0]["col"]
                            bca = dls[:, c0 : c0 + k].to_broadcast([P, k, P])
                            iap = iot[:, :]
                            iota_b = bass.AP(
                                iap.tensor, iap.offset,
                                [list(iap.ap[0]), [0, k], list(iap.ap[1])],
                            )
                            nc.vector.tensor_tensor(
                                out=st[:, : k * P].rearrange(
                                    "p (a q) -> p a q", q=P),
                                in0=bca, in1=iota_b,
                                op=mybir.AluOpType.is_equal,
                            )
                            for j, sd in enumerate(grp):
                                s = sd["slot"]
                                if s not in psums:
                                    psums[s] = pp.tile([P, Fdim], f32,
                                                       tag="agg", name="aggps")
                                nc.tensor.matmul(
                                    out=psums[s][:],
                                    lhsT=st[:, j * P : (j + 1) * P],
                                    rhs=gb[:, sd["b"] * Fdim :
                                           (sd["b"] + 1) * Fdim],
                                    start=sd["start"], stop=sd["stop"],
                                )
                                if sd["stop"]:
                                    epilogue(sd["slot"], psums[sd["slot"]])
                    for s in sup["slots"]:
                        if s not in psums:
                            psz = pp.tile([P, Fdim], f32, tag="agg")
                            nc.vector.memset(psz[:], 0.0)
                            epilogue(s, psz)

            def epi1(s, ps):
                ownt = op.tile([P, H], f32, tag="own1")
                nc.sync.dma_start(out=ownt[:],
                                  in_=htab[s * P : (s + 1) * P, :])
                t1 = ep.tile([P, H], f32, tag="e1")
                nc.vector.tensor_add(out=t1[:], in0=ps[:], in1=ownt[:])
                t2 = ep.tile([P, H], f32, tag="e2")
                nc.vector.tensor_tensor(
                    out=t2[:], in0=t1[:],
                    in1=ddst[:, s : s + 1].to_broadcast([P, H]),
                    op=mybir.AluOpType.mult,
                )
                t3 = ep.tile([P, H], f32, tag="e3")
                nc.vector.tensor_add(out=t3[:], in0=t2[:], in1=b1s[:])
                h1 = ep.tile([P, H], f32, tag="h1")
                nc.scalar.activation(
                    out=h1[:], in_=t3[:],
                    func=mybir.ActivationFunctionType.Relu,
                )
                nc.sync.dma_start(out=t_emb[s * P : (s + 1) * P, :],
                                  in_=h1[:])
                pt = pp.tile([P, P], f32, tag="ptr", bufs=2)
                nc.tensor.transpose(out=pt[:], in_=h1[:], identity=ident[:])
                h1T = ep.tile([P, P], f32, tag="h1T")
                nc.vector.tensor_copy(out=h1T[:], in_=pt[:])
                pz = pp.tile([P, C], f32, tag="pz", bufs=2)
                nc.tensor.matmul(out=pz[:], lhsT=h1T[:], rhs=W2s[:],
                                 start=True, stop=True)
                zt = ep.tile([P, C], f32, tag="zt")
                nc.vector.tensor_tensor(
                    out=zt[:], in0=pz[:],
                    in1=ddst[:, s : s + 1].to_broadcast([P, C]),
                    op=mybir.AluOpType.mult,
                )
                nc.sync.dma_start(out=zb[s * P : (s + 1) * P, :], in_=zt[:])

            agg_layer(plan["meta1"], dl1s, ix1s, htab, H, epi1)

            nc.gpsimd.collective_compute(
                "AllGather",
                mybir.AluOpType.bypass,
                replica_groups=[list(range(NCORES))],
                ins=[zb.opt()],
                outs=[ztab.opt()],
            )

            def epi2(s, ps):
                zown = op.tile([P, C], f32, tag="own2")
                nc.sync.dma_start(out=zown[:], in_=zb[s * P : (s + 1) * P, :])
                t1 = ep.tile([P, C], f32, tag="f1")
                nc.vector.tensor_add(out=t1[:], in0=ps[:], in1=zown[:])
                t2 = ep.tile([P, C], f32, tag="f2")
                nc.vector.tensor_tensor(
                    out=t2[:], in0=t1[:],
                    in1=ddst[:, s : s + 1].to_broadcast([P, C]),
                    op=mybir.AluOpType.mult,
                )
                t3 = ep.tile([P, C], f32, tag="f3")
                nc.vector.tensor_add(out=t3[:], in0=t2[:], in1=b2s[:])
                mx = ep.tile([P, 1], f32, tag="mx")
                nc.vector.reduce_max(out=mx[:], in_=t3[:],
                                     axis=mybir.AxisListType.X)
                xcc = ep.tile([P, C], f32, tag="xcc")
                nc.vector.tensor_tensor(
                    out=xcc[:], in0=t3[:], in1=mx[:].to_broadcast([P, C]),
                    op=mybir.AluOpType.subtract,
                )
                exv = ep.tile([P, C], f32, tag="exv")
                smv = ep.tile([P, 1], f32, tag="smv")
                nc.scalar.activation(
                    out=exv[:], in_=xcc[:],
                    func=mybir.ActivationFunctionType.Exp,
                    accum_out=smv[:],
                )
                lsv = ep.tile([P, 1], f32, tag="lsv")
                nc.scalar.activation(
                    out=lsv[:], in_=smv[:],
                    func=mybir.ActivationFunctionType.Ln,
                )
                ov = ep.tile([P, C], f32, tag="ov")
                nc.vector.tensor_tensor(
                    out=ov[:], in0=xcc[:], in1=lsv[:].to_broadcast([P, C]),
                    op=mybir.AluOpType.subtract,
                )
                nc.sync.dma_start(out=t_logp[s * P : (s + 1) * P, :],
                                  in_=ov[:])

            agg_layer(plan["meta2"], dl2s, ix2s, ztab, C, epi2)

    nc.compile()
    return nc


def _in_maps(plan):
    maps = []
    for c in range(NCORES):
        pc = plan["percore"][c]
        maps.append({
            "xT": pc["xT"], "W1": pc["W1"], "W2": pc["W2"],
            "b1b": pc["b1b"], "b2b": pc["b2b"], "iota": pc["iota"],
            "dinvsrc": pc["dinvsrc"], "dinvdst": pc["dinvdst"],
            "dl1": pc["dl1"], "ix1": pc["ix1"],
            "dl2": pc["dl2"], "ix2": pc["ix2"],
        })
    return maps


def _assemble(plan, results):
    N, H, C, TPC = plan["N"], plan["H"], plan["C"], plan["TPC"]
    NP = plan["NP"]
    tile_at = plan["tile_at"]
    emb = np.zeros((NP, H), np.float32)
    logp = np.zeros((NP, C), np.float32)
    for c in range(NCORES):
        e = results[c]["emb"]
        l = results[c]["logp"]
        for s in range(TPC):
            gt = tile_at[s, c]
            emb[gt * P : (gt + 1) * P] = e[s * P : (s + 1) * P]
            logp[gt * P : (gt + 1) * P] = l[s * P : (s + 1) * P]
    return logp[:N], emb[:N]


def kernel(x, W1, b1, W2, b2, edge_index, _trace=False, _want_time=False):
    plan = _plan(np.asarray(x), np.asarray(W1), np.asarray(b1),
                 np.asarray(W2), np.asarray(b2), np.asarray(edge_index))
    nc = _build(plan)
    res = run_bass_kernel_spmd(nc, _in_maps(plan),
                               core_ids=list(range(NCORES)), trace=_trace)
    logp, emb = _assemble(plan, res.results)
    if _want_time:
        return (logp, emb), res.exec_time_ns
    return logp, emb


# revision 7
# speedup vs baseline: 1.0217x; 1.0217x over previous
"""GCN (2-layer, symmetric-normalized adjacency) on 8 TRN2 NeuronCores.

Strategy:
  - Full f32. Host does graph preprocessing only (normalization constants,
    partitioning, index tables); all FLOPs on x/W run on device.
  - Nodes padded to NP; dst tiles of 128 nodes; tiles assigned to cores with
    count balancing (sort by edge count, deal round-robin).
  - GEMM1 per-core computes h' = dinv[src]*(x@W1) for ALL nodes into a DRAM
    table (per-core column order: own tiles first, so self-loop rows sit at
    static offsets).
  - Edge messages gathered from the table with bulk dma_gather (int16
    indices -> 4 src-row chunks; per-(tile,chunk) quotas padded to the max
    across cores so the program is SPMD-uniform).
  - Aggregation = one-hot matmuls on the TensorEngine: S[p,j] =
    (dstloc[p]==j), PSUM-accumulated per dst tile; self-loop row added from
    the table; epilogue relu(dinv*(agg+own)+b1).
  - z' = dinv*(h1@W2) shards AllGathered into a z table; layer 2 repeats the
    gather/aggregate structure and ends in log_softmax.
"""

import sys
import types

import numpy as np


def _install_ntff_hook():
    if "antenv.axon_hooks" in sys.modules:
        return
    try:
        from trn_agent_boot.trn_boot import _ntff_profile_via_ctypes

        hook = _ntff_profile_via_ctypes("/opt/axon/libaxon_pjrt.so")
    except Exception:
        hook = None
    mod = types.ModuleType("antenv.axon_hooks")
    mod.get_axon_ntff_profile_hook = lambda: hook
    mod.set_axon_ntff_profile_hook = lambda h: None
    sys.modules["antenv.axon_hooks"] = mod


_install_ntff_hook()

import concourse.bass as bass
import concourse.bacc as bacc
import concourse.tile as tile
import concourse.mybir as mybir
from concourse.bass_utils import run_bass_kernel_spmd
from concourse.masks import make_identity

P = 128
NCORES = 8
NCHUNK = 4
G = 4        # dst tiles per superbatch
SG = 16      # one-hot columns per S tile
XB = 4       # GEMM1 tiles per staging/DMA batch
INVALID = 1000.0


def _round_up(x, m):
    return -(-x // m) * m


def _build_layer_schedule(lists, q16, supers, TPC):
    """Static (core-uniform) schedule for one layer + per-core dstloc/idx.

    lists[c][s][ch] = (rows_in_chunk, dst_local) per core/slot/chunk,
    q16[s, ch] = padded (max-over-cores) count, multiple of 16.
    """
    meta = {"supers": []}
    dl_cols = []     # column fill instructions: (col, seg info)
    col_count = 0
    idx16_count = 0
    idxw = [[] for _ in range(NCORES)]

    for slots in supers:
        calls = []
        segs = []    # (slot, start_pos, qlen, chunk)
        pos = 0
        for ch in range(NCHUNK):
            q_sum = int(sum(q16[s, ch] for s in slots))
            if q_sum == 0:
                continue
            Q = _round_up(q_sum, P)
            calls.append({"chunk": ch, "Q": Q, "i16": idx16_count,
                          "blk0": pos // P})
            for c in range(NCORES):
                arr = np.zeros(Q, np.int64)
                o = 0
                for s in slots:
                    q = int(q16[s, ch])
                    if q == 0:
                        continue
                    rows = lists[c][s][ch][0]
                    arr[o : o + len(rows)] = rows
                    o += q
                w = arr.astype(np.int16).reshape(Q // 16, 16).T
                idxw[c].append(np.tile(w, (NCORES, 1)))
            for s in slots:
                q = int(q16[s, ch])
                if q:
                    segs.append((s, pos, q, ch))
                    pos += q
            pos += Q - q_sum
            idx16_count += Q // 16
        M = pos
        assert M % P == 0
        nb = M // P
        sides = []
        slot_sides = {}
        for b in range(nb):
            lo, hi = b * P, (b + 1) * P
            for (s, sp, ln, ch) in segs:
                if sp < hi and sp + ln > lo:
                    sd = {"b": b, "col": col_count, "slot": s,
                          "sp": sp, "ln": ln, "chunk": ch}
                    sides.append(sd)
                    slot_sides.setdefault(s, []).append(sd)
                    col_count += 1
        for s, sl in slot_sides.items():
            for sd in sl:
                sd["start"] = sd is sl[0]
                sd["stop"] = sd is sl[-1]
        meta["supers"].append({"slots": slots, "calls": calls, "nb": nb,
                               "sides": sides})
    meta["ncols"] = max(col_count, 1)
    meta["nidx16"] = max(idx16_count, 1)

    dl = [np.full((P, meta["ncols"]), INVALID, np.float32)
          for _ in range(NCORES)]
    for sup in meta["supers"]:
        for sd in sup["sides"]:
            sp, ln, s, ch = sd["sp"], sd["ln"], sd["slot"], sd["chunk"]
            lo, hi = sd["b"] * P, (sd["b"] + 1) * P
            a = max(sp, lo)
            b_ = min(sp + ln, hi)
            r0 = a - sp
            for c in range(NCORES):
                ed = lists[c][s][ch][1]
                r1 = min(b_ - sp, len(ed))
                if r1 > r0:
                    pidx = (a - lo) + np.arange(r1 - r0)
                    dl[c][pidx, sd["col"]] = ed[r0:r1]

    idxcat = []
    for c in range(NCORES):
        if idxw[c]:
            idxcat.append(np.ascontiguousarray(
                np.concatenate(idxw[c], axis=1)))
        else:
            idxcat.append(np.zeros((P, 1), np.int16))
    return meta, dl, idxcat


def _plan(x, W1, b1, W2, b2, edge_index):
    N, F_IN = x.shape
    H = W1.shape[1]
    C = W2.shape[1]
    src = np.asarray(edge_index[0], dtype=np.int64)
    dst = np.asarray(edge_index[1], dtype=np.int64)

    NP = _round_up(N, P * NCORES)
    T = NP // P
    TPC = T // NCORES
    CH = NP // NCHUNK
    assert CH <= 32767 and CH % P == 0

    deg = np.bincount(dst, minlength=NP).astype(np.float64) + 1.0
    dinv = (1.0 / np.sqrt(deg)).astype(np.float32)
    dinv[N:] = 0.0

    etile = dst // P
    counts = np.bincount(etile, minlength=T)
    order = np.argsort(-counts, kind="stable")
    tile_at = order.reshape(TPC, NCORES)            # [slot, core] -> tile
    tile_core = np.empty(T, np.int64)
    tile_slot = np.empty(T, np.int64)
    tile_core[tile_at.reshape(-1)] = np.tile(np.arange(NCORES), TPC)
    tile_slot[tile_at.reshape(-1)] = np.repeat(np.arange(TPC), NCORES)

    nodes = np.arange(NP, dtype=np.int64)
    qbase, qrem = TPC // NCHUNK, TPC % NCHUNK
    qsizes = [qbase + (1 if j < qrem else 0) for j in range(NCHUNK)]
    qstart = np.cumsum([0] + qsizes)
    qof = np.concatenate([np.full(qsizes[j], j, np.int64)
                          for j in range(NCHUNK)]) if TPC else np.zeros(0, np.int64)
    nslot = tile_slot[nodes // P]
    ncore = tile_core[nodes // P]
    nq = qof[nslot]
    zchunk = nq
    zlocal = (ncore * np.array(qsizes)[nq] * P
              + (nslot - qstart[nq]) * P + nodes % P)

    hrow = []
    colnode = []
    for c in range(NCORES):
        own = list(tile_at[:, c])
        rest = [t for t in range(T) if tile_core[t] != c]
        sq = np.array(own + rest, dtype=np.int64)
        pos = np.empty(T, np.int64)
        pos[sq] = np.arange(T)
        hrow.append(pos[nodes // P] * P + nodes % P)
        colnode.append((sq[:, None] * P + np.arange(P)[None, :]).reshape(-1))

    ecore = tile_core[etile]
    eslot = tile_slot[etile]
    edloc = dst % P
    supers = [list(range(g, min(g + G, TPC))) for g in range(0, TPC, G)]

    def edge_lists(erow_per_core, echunk_per_core):
        lists = [[[None] * NCHUNK for _ in range(TPC)] for _ in range(NCORES)]
        for c in range(NCORES):
            m = ecore == c
            er = erow_per_core[c][m]
            es = eslot[m]
            ed = edloc[m]
            ech = echunk_per_core[c][m]
            key = np.lexsort((ed, ech, es))
            er, es, ed, ech = er[key], es[key], ed[key], ech[key]
            grp = es * NCHUNK + ech
            bounds = np.searchsorted(grp, np.arange(TPC * NCHUNK + 1))
            for s in range(TPC):
                for ch in range(NCHUNK):
                    a, b = bounds[s * NCHUNK + ch], bounds[s * NCHUNK + ch + 1]
                    lists[c][s][ch] = (er[a:b], ed[a:b])
        q16 = np.zeros((TPC, NCHUNK), np.int64)
        for s in range(TPC):
            for ch in range(NCHUNK):
                mx = max(len(lists[c][s][ch][0]) for c in range(NCORES))
                q16[s, ch] = _round_up(mx, 16) if mx else 0
        return lists, q16

    l1, q1 = edge_lists([hrow[c][src] % CH for c in range(NCORES)],
                        [hrow[c][src] // CH for c in range(NCORES)])
    meta1, dl1, ix1 = _build_layer_schedule(l1, q1, supers, TPC)
    l2, q2 = edge_lists([zlocal[src] for _ in range(NCORES)],
                        [zchunk[src] for _ in range(NCORES)])
    meta2, dl2, ix2 = _build_layer_schedule(l2, q2, supers, TPC)

    plan = {
        "N": N, "F_IN": F_IN, "H": H, "C": C, "NP": NP, "T": T, "TPC": TPC,
        "CH": CH, "meta1": meta1, "meta2": meta2, "tile_at": tile_at,
        "qsizes": qsizes, "qstart": list(qstart), "qof": list(qof),
    }
    xpad = np.zeros((NP, F_IN), np.float32)
    xpad[:N] = np.asarray(x, np.float32)
    percore = []
    for c in range(NCORES):
        xT = np.ascontiguousarray(xpad[colnode[c]].T)
        dinvsrc = np.ascontiguousarray(dinv[colnode[c]].reshape(T, P).T)
        ownnodes = (tile_at[:, c][:, None] * P +
                    np.arange(P)[None, :]).reshape(-1)
        dinvdst = np.ascontiguousarray(dinv[ownnodes].reshape(TPC, P).T)
        percore.append({
            "xT": xT,
            "W1": np.asarray(W1, np.float32),
            "W2": np.asarray(W2, np.float32),
            "b1b": np.tile(np.asarray(b1, np.float32)[None, :], (P, 1)),
            "b2b": np.tile(np.asarray(b2, np.float32)[None, :], (P, 1)),
            "iota": np.tile(np.arange(P, dtype=np.float32)[None, :], (P, 1)),
            "dinvsrc": dinvsrc,
            "dinvdst": dinvdst,
            "dl1": dl1[c], "ix1": ix1[c],
            "dl2": dl2[c], "ix2": ix2[c],
        })
    plan["percore"] = percore
    return plan


def _build(plan):
    F_IN, H, C = plan["F_IN"], plan["H"], plan["C"]
    NP, T, TPC, CH = plan["NP"], plan["T"], plan["TPC"], plan["CH"]
    pc0 = plan["percore"][0]
    f32 = mybir.dt.float32

    nc = bacc.Bacc("TRN2", target_bir_lowering=False, debug=False,
                   num_devices=NCORES)
    t_xT = nc.dram_tensor("xT", [F_IN, NP], f32, kind="ExternalInput")
    t_W1 = nc.dram_tensor("W1", [F_IN, H], f32, kind="ExternalInput")
    t_W2 = nc.dram_tensor("W2", [H, C], f32, kind="ExternalInput")
    t_b1b = nc.dram_tensor("b1b", [P, H], f32, kind="ExternalInput")
    t_b2b = nc.dram_tensor("b2b", [P, C], f32, kind="ExternalInput")
    t_iota = nc.dram_tensor("iota", [P, P], f32, kind="ExternalInput")
    t_dsrc = nc.dram_tensor("dinvsrc", [P, T], f32, kind="ExternalInput")
    t_ddst = nc.dram_tensor("dinvdst", [P, TPC], f32, kind="ExternalInput")
    t_dl1 = nc.dram_tensor("dl1", list(pc0["dl1"].shape), f32,
                           kind="ExternalInput")
    t_ix1 = nc.dram_tensor("ix1", list(pc0["ix1"].shape), mybir.dt.int16,
                           kind="ExternalInput")
    t_dl2 = nc.dram_tensor("dl2", list(pc0["dl2"].shape), f32,
                           kind="ExternalInput")
    t_ix2 = nc.dram_tensor("ix2", list(pc0["ix2"].shape), mybir.dt.int16,
                           kind="ExternalInput")
    t_emb = nc.dram_tensor("emb", [TPC * P, H], f32, kind="ExternalOutput")
    t_logp = nc.dram_tensor("logp", [TPC * P, C], f32, kind="ExternalOutput")

    with tile.TileContext(nc) as tc:
        with (
            tc.tile_pool(name="const", bufs=1) as cp,
            tc.tile_pool(name="xin", bufs=2) as xp,
            tc.tile_pool(name="hst", bufs=3) as hp,
            tc.tile_pool(name="gbuf", bufs=2) as gp,
            tc.tile_pool(name="spool", bufs=3) as sp,
            tc.tile_pool(name="epool", bufs=4) as ep,
            tc.tile_pool(name="opool", bufs=3) as op,
            tc.tile_pool(name="psagg", bufs=G, space="PSUM") as pp,
            tc.tile_pool(name="dram", bufs=1, space="DRAM") as dp,
        ):
            def load_const(t, shape, nm, dtype=f32):
                s = cp.tile(shape, dtype, name=nm, tag=nm)
                nc.sync.dma_start(out=s[:], in_=t[:, :])
                return s

            W1s = load_const(t_W1, [F_IN, H], "cW1")
            W2s = load_const(t_W2, [H, C], "cW2")
            b1s = load_const(t_b1b, [P, H], "cb1")
            b2s = load_const(t_b2b, [P, C], "cb2")
            iot = load_const(t_iota, [P, P], "ciota")
            dsrc = load_const(t_dsrc, [P, T], "cdsrc")
            ddst = load_const(t_ddst, [P, TPC], "cddst")
            dl1s = load_const(t_dl1, list(pc0["dl1"].shape), "cdl1")
            ix1s = load_const(t_ix1, list(pc0["ix1"].shape), "cix1", mybir.dt.int16)
            dl2s = load_const(t_dl2, list(pc0["dl2"].shape), "cdl2")
            ix2s = load_const(t_ix2, list(pc0["ix2"].shape), "cix2", mybir.dt.int16)
            ident = cp.tile([P, P], f32)
            make_identity(nc, ident[:])

            htabs = [dp.tile([CH, H], f32, name=f"htab{i}", tag=f"htab{i}")
                     for i in range(NCHUNK)]
            qsizes, qstart = plan["qsizes"], plan["qstart"]
            qof = plan["qof"]
            zbs = [dp.tile([max(qsizes[j], 1) * P, C], f32, name=f"zb{j}",
                           tag=f"zb{j}") for j in range(NCHUNK)]
            ztabs = [dp.tile([max(qsizes[j], 1) * NCORES * P, C], f32,
                             name=f"ztab{j}", tag=f"ztab{j}")
                     for j in range(NCHUNK)]

            # ---- Phase A: h' table
            XCW = 28 * P if NP % (28 * P) == 0 else P
            ntc = XCW // P
            for x0 in range(0, NP, XCW):
                xc = xp.tile([F_IN, XCW], f32, tag="xc")
                nc.sync.dma_start(out=xc[:], in_=t_xT[:, x0 : x0 + XCW])
                for jj in range(0, ntc, XB):
                    nbh = min(XB, ntc - jj)
                    hs = hp.tile([P, XB * H], f32, tag="hs")
                    for k in range(nbh):
                        gt = (x0 // P) + jj + k
                        psA = pp.tile([P, H], f32, tag="agg")
                        nc.tensor.matmul(
                            out=psA[:],
                            lhsT=xc[:, (jj + k) * P : (jj + k + 1) * P],
                            rhs=W1s[:],
                            start=True, stop=True,
                        )
                        nc.vector.tensor_tensor(
                            out=hs[:, k * H : (k + 1) * H], in0=psA[:],
                            in1=dsrc[:, gt : gt + 1].to_broadcast([P, H]),
                            op=mybir.AluOpType.mult,
                        )
                    r0 = (x0 // P + jj) * P
                    chn, rloc = r0 // CH, r0 % CH
                    nc.sync.dma_start(
                        out=htabs[chn][rloc : rloc + nbh * P, :].rearrange(
                            "(a p) h -> p a h", p=P),
                        in_=hs[:, : nbh * H].rearrange("p (a h) -> p a h", h=H),
                    )

            def agg_layer(meta, dls, ixs, table, Fdim, epilogue):
                for sup in meta["supers"]:
                    nb = sup["nb"]
                    psums = {}
                    if nb:
                        gb = gp.tile([P, nb * Fdim], f32, tag="gb")
                        for call in sup["calls"]:
                            ch, Q = call["chunk"], call["Q"]
                            d0 = call["blk0"] * Fdim
                            nc.gpsimd.dma_gather(
                                gb[:, d0 : d0 + (Q // P) * Fdim].rearrange(
                                    "p (a h) -> p a h", h=Fdim),
                                table[ch][:, :] if isinstance(table, list)
                                else table[ch * CH : (ch + 1) * CH, :],
                                ixs[:, call["i16"] : call["i16"] + Q // 16],
                                Q, Q, Fdim,
                                single_packet=False,
                            )
                        sides = sup["sides"]
                        for i0 in range(0, len(sides), SG):
                            grp = sides[i0 : i0 + SG]
                            k = len(grp)
                            st = sp.tile([P, SG * P], f32, tag="s")
                            c0 = grp[0]["col"]
                            bca = dls[:, c0 : c0 + k].to_broadcast([P, k, P])
                            iap = iot[:, :]
                            iota_b = bass.AP(
                                iap.tensor, iap.offset,
                                [list(iap.ap[0]), [0, k], list(iap.ap[1])],
                            )
                            nc.vector.tensor_tensor(
                                out=st[:, : k * P].rearrange(
                                    "p (a q) -> p a q", q=P),
                                in0=bca, in1=iota_b,
                                op=mybir.AluOpType.is_equal,
                            )
                            for j, sd in enumerate(grp):
                                s = sd["slot"]
                                if s not in psums:
                                    psums[s] = pp.tile([P, Fdim], f32,
                                                       tag="agg", name="aggps")
                                nc.tensor.matmul(
                                    out=psums[s][:],
                                    lhsT=st[:, j * P : (j + 1) * P],
                                    rhs=gb[:, sd["b"] * Fdim :
                                           (sd["b"] + 1) * Fdim],
                                    start=sd["start"], stop=sd["stop"],
                                )
                                if sd["stop"]:
                                    epilogue(sd["slot"], psums[sd["slot"]])
                    for s in sup["slots"]:
                        if s not in psums:
                            psz = pp.tile([P, Fdim], f32, tag="agg")
                            nc.vector.memset(psz[:], 0.0)
                            epilogue(s, psz)

            def epi1(s, ps):
                ownt = op.tile([P, H], f32, tag="own1")
                nc.sync.dma_start(out=ownt[:],
                                  in_=htabs[0][s * P : (s + 1) * P, :])
                t1 = ep.tile([P, H], f32, tag="e1")
                nc.vector.tensor_add(out=t1[:], in0=ps[:], in1=ownt[:])
                t2 = ep.tile([P, H], f32, tag="e2")
                nc.vector.tensor_tensor(
                    out=t2[:], in0=t1[:],
                    in1=ddst[:, s : s + 1].to_broadcast([P, H]),
                    op=mybir.AluOpType.mult,
                )
                t3 = ep.tile([P, H], f32, tag="e3")
                nc.vector.tensor_add(out=t3[:], in0=t2[:], in1=b1s[:])
                h1 = ep.tile([P, H], f32, tag="h1")
                nc.scalar.activation(
                    out=h1[:], in_=t3[:],
                    func=mybir.ActivationFunctionType.Relu,
                )
                nc.sync.dma_start(out=t_emb[s * P : (s + 1) * P, :],
                                  in_=h1[:])
                pt = pp.tile([P, P], f32, tag="ptr", bufs=2)
                nc.tensor.transpose(out=pt[:], in_=h1[:], identity=ident[:])
                h1T = ep.tile([P, P], f32, tag="h1T")
                nc.vector.tensor_copy(out=h1T[:], in_=pt[:])
                pz = pp.tile([P, C], f32, tag="pz", bufs=2)
                nc.tensor.matmul(out=pz[:], lhsT=h1T[:], rhs=W2s[:],
                                 start=True, stop=True)
                zt = ep.tile([P, C], f32, tag="zt")
                nc.vector.tensor_tensor(
                    out=zt[:], in0=pz[:],
                    in1=ddst[:, s : s + 1].to_broadcast([P, C]),
                    op=mybir.AluOpType.mult,
                )
                j = qof[s]
                r = (s - qstart[j]) * P
                nc.sync.dma_start(out=zbs[j][r : r + P, :], in_=zt[:])

            agg_layer(plan["meta1"], dl1s, ix1s, htabs, H, epi1)

            for j in range(NCHUNK):
                if qsizes[j]:
                    nc.gpsimd.collective_compute(
                        "AllGather",
                        mybir.AluOpType.bypass,
                        replica_groups=[list(range(NCORES))],
                        ins=[zbs[j].opt()],
                        outs=[ztabs[j].opt()],
                    )

            def epi2(s, ps):
                zown = op.tile([P, C], f32, tag="own2")
                j = qof[s]
                r = (s - qstart[j]) * P
                nc.sync.dma_start(out=zown[:], in_=zbs[j][r : r + P, :])
                t1 = ep.tile([P, C], f32, tag="f1")
                nc.vector.tensor_add(out=t1[:], in0=ps[:], in1=zown[:])
                t2 = ep.tile([P, C], f32, tag="f2")
                nc.vector.tensor_tensor(
                    out=t2[:], in0=t1[:],
                    in1=ddst[:, s : s + 1].to_broadcast([P, C]),
                    op=mybir.AluOpType.mult,
                )
                t3 = ep.tile([P, C], f32, tag="f3")
                nc.vector.tensor_add(out=t3[:], in0=t2[:], in1=b2s[:])
                mx = ep.tile([P, 1], f32, tag="mx")
                nc.vector.reduce_max(out=mx[:], in_=t3[:],
                                     axis=mybir.AxisListType.X)
                xcc = ep.tile([P, C], f32, tag="xcc")
                nc.vector.tensor_tensor(
                    out=xcc[:], in0=t3[:], in1=mx[:].to_broadcast([P, C]),
                    op=mybir.AluOpType.subtract,
                )
                exv = ep.tile([P, C], f32, tag="exv")
                smv = ep.tile([P, 1], f32, tag="smv")
                nc.scalar.activation(
                    out=exv[:], in_=xcc[:],
                    func=mybir.ActivationFunctionType.Exp,
                    accum_out=smv[:],
                )
                lsv = ep.tile([P, 1], f32, tag="lsv")
                nc.scalar.activation(
                    out=lsv[:], in_=smv[:],
                    func=mybir.ActivationFunctionType.Ln,
                )
                ov = ep.tile([P, C], f32, tag="ov")
                nc.vector.tensor_tensor(
                    out=ov[:], in0=xcc[:], in1=lsv[:].to_broadcast([P, C]),
                    op=mybir.AluOpType.subtract,
                )
                nc.sync.dma_start(out=t_logp[s * P : (s + 1) * P, :],
                                  in_=ov[:])

            agg_layer(plan["meta2"], dl2s, ix2s, ztabs, C, epi2)

    nc.compile()
    return nc


def _in_maps(plan):
    maps = []
    for c in range(NCORES):
        pc = plan["percore"][c]
        maps.append({
            "xT": pc["xT"], "W1": pc["W1"], "W2": pc["W2"],
            "b1b": pc["b1b"], "b2b": pc["b2b"], "iota": pc["iota"],
            "dinvsrc": pc["dinvsrc"], "dinvdst": pc["dinvdst"],
            "dl1": pc["dl1"], "ix1": pc["ix1"],
            "dl2": pc["dl2"], "ix2": pc["ix2"],
        })
    return maps


def _assemble(plan, results):
    N, H, C, TPC = plan["N"], plan["H"], plan["C"], plan["TPC"]
    NP = plan["NP"]
    tile_at = plan["tile_at"]
    emb = np.zeros((NP, H), np.float32)
    logp = np.zeros((NP, C), np.float32)
    for c in range(NCORES):
        e = results[c]["emb"]
        l = results[c]["logp"]
        for s in range(TPC):
            gt = tile_at[s, c]
            emb[gt * P : (gt + 1) * P] = e[s * P : (s + 1) * P]
            logp[gt * P : (gt + 1) * P] = l[s * P : (s + 1) * P]
    return logp[:N], emb[:N]


def kernel(x, W1, b1, W2, b2, edge_index, _trace=False, _want_time=False):
    plan = _plan(np.asarray(x), np.asarray(W1), np.asarray(b1),
                 np.asarray(W2), np.asarray(b2), np.asarray(edge_index))
    nc = _build(plan)
    res = run_bass_kernel_spmd(nc, _in_maps(plan),
                               core_ids=list(range(NCORES)), trace=_trace)
    logp, emb = _assemble(plan, res.results)
    if _want_time:
        return (logp, emb), res.exec_time_ns
    return logp, emb


# revision 8
# speedup vs baseline: 1.0390x; 1.0169x over previous
"""GCN (2-layer, symmetric-normalized adjacency) on 8 TRN2 NeuronCores.

Strategy:
  - Full f32. Host does graph preprocessing only (normalization constants,
    partitioning, index tables); all FLOPs on x/W run on device.
  - Nodes padded to NP; dst tiles of 128 nodes; tiles assigned to cores with
    count balancing (sort by edge count, deal round-robin).
  - GEMM1 per-core computes h' = dinv[src]*(x@W1) for ALL nodes into a DRAM
    table (per-core column order: own tiles first, so self-loop rows sit at
    static offsets).
  - Edge messages gathered from the table with bulk dma_gather (int16
    indices -> 4 src-row chunks; per-(tile,chunk) quotas padded to the max
    across cores so the program is SPMD-uniform).
  - Aggregation = one-hot matmuls on the TensorEngine: S[p,j] =
    (dstloc[p]==j), PSUM-accumulated per dst tile; self-loop row added from
    the table; epilogue relu(dinv*(agg+own)+b1).
  - z' = dinv*(h1@W2) shards AllGathered into a z table; layer 2 repeats the
    gather/aggregate structure and ends in log_softmax.
"""

import sys
import types

import numpy as np


def _install_ntff_hook():
    if "antenv.axon_hooks" in sys.modules:
        return
    try:
        from trn_agent_boot.trn_boot import _ntff_profile_via_ctypes

        hook = _ntff_profile_via_ctypes("/opt/axon/libaxon_pjrt.so")
    except Exception:
        hook = None
    mod = types.ModuleType("antenv.axon_hooks")
    mod.get_axon_ntff_profile_hook = lambda: hook
    mod.set_axon_ntff_profile_hook = lambda h: None
    sys.modules["antenv.axon_hooks"] = mod


_install_ntff_hook()

import concourse.bass as bass
import concourse.bacc as bacc
import concourse.tile as tile
import concourse.mybir as mybir
from concourse.bass_utils import run_bass_kernel_spmd
from concourse.masks import make_identity

P = 128
NCORES = 8
NCHUNK = 4
G = 4        # dst tiles per superbatch
SG = 16      # one-hot columns per S tile
XB = 4       # GEMM1 tiles per staging/DMA batch
INVALID = 1000.0


def _round_up(x, m):
    return -(-x // m) * m


def _build_layer_schedule(lists, q16, supers, TPC):
    """Static (core-uniform) schedule for one layer + per-core dstloc/idx.

    lists[c][s][ch] = (rows_in_chunk, dst_local) per core/slot/chunk,
    q16[s, ch] = padded (max-over-cores) count, multiple of 16.
    """
    meta = {"supers": []}
    dl_cols = []     # column fill instructions: (col, seg info)
    col_count = 0
    idx16_count = 0
    idxw = [[] for _ in range(NCORES)]

    for slots in supers:
        calls = []
        segs = []    # (slot, start_pos, qlen, chunk)
        pos = 0
        for ch in range(NCHUNK):
            q_sum = int(sum(q16[s, ch] for s in slots))
            if q_sum == 0:
                continue
            Q = _round_up(q_sum, P)
            calls.append({"chunk": ch, "Q": Q, "i16": idx16_count,
                          "blk0": pos // P})
            for c in range(NCORES):
                arr = np.zeros(Q, np.int64)
                o = 0
                for s in slots:
                    q = int(q16[s, ch])
                    if q == 0:
                        continue
                    rows = lists[c][s][ch][0]
                    arr[o : o + len(rows)] = rows
                    o += q
                w = arr.astype(np.int16).reshape(Q // 16, 16).T
                idxw[c].append(np.tile(w, (NCORES, 1)))
            for s in slots:
                q = int(q16[s, ch])
                if q:
                    segs.append((s, pos, q, ch))
                    pos += q
            pos += Q - q_sum
            idx16_count += Q // 16
        M = pos
        assert M % P == 0
        nb = M // P
        sides = []
        slot_sides = {}
        for b in range(nb):
            lo, hi = b * P, (b + 1) * P
            for (s, sp, ln, ch) in segs:
                if sp < hi and sp + ln > lo:
                    sd = {"b": b, "col": col_count, "slot": s,
                          "sp": sp, "ln": ln, "chunk": ch}
                    sides.append(sd)
                    slot_sides.setdefault(s, []).append(sd)
                    col_count += 1
        for s, sl in slot_sides.items():
            for sd in sl:
                sd["start"] = sd is sl[0]
                sd["stop"] = sd is sl[-1]
        meta["supers"].append({"slots": slots, "calls": calls, "nb": nb,
                               "sides": sides})
    meta["ncols"] = max(col_count, 1)
    meta["nidx16"] = max(idx16_count, 1)

    dl = [np.full((P, meta["ncols"]), INVALID, np.float32)
          for _ in range(NCORES)]
    for sup in meta["supers"]:
        for sd in sup["sides"]:
            sp, ln, s, ch = sd["sp"], sd["ln"], sd["slot"], sd["chunk"]
            lo, hi = sd["b"] * P, (sd["b"] + 1) * P
            a = max(sp, lo)
            b_ = min(sp + ln, hi)
            r0 = a - sp
            for c in range(NCORES):
                ed = lists[c][s][ch][1]
                r1 = min(b_ - sp, len(ed))
                if r1 > r0:
                    pidx = (a - lo) + np.arange(r1 - r0)
                    dl[c][pidx, sd["col"]] = ed[r0:r1]

    idxcat = []
    for c in range(NCORES):
        if idxw[c]:
            idxcat.append(np.ascontiguousarray(
                np.concatenate(idxw[c], axis=1)))
        else:
            idxcat.append(np.zeros((P, 1), np.int16))
    return meta, dl, idxcat


def _plan(x, W1, b1, W2, b2, edge_index):
    N, F_IN = x.shape
    H = W1.shape[1]
    C = W2.shape[1]
    src = np.asarray(edge_index[0], dtype=np.int64)
    dst = np.asarray(edge_index[1], dtype=np.int64)

    NP = _round_up(N, P * NCORES)
    T = NP // P
    TPC = T // NCORES
    CH = NP // NCHUNK
    assert CH <= 32767 and CH % P == 0

    deg = np.bincount(dst, minlength=NP).astype(np.float64) + 1.0
    dinv = (1.0 / np.sqrt(deg)).astype(np.float32)
    dinv[N:] = 0.0

    etile = dst // P
    counts = np.bincount(etile, minlength=T)
    order = np.argsort(-counts, kind="stable")
    tile_at = order.reshape(TPC, NCORES)            # [slot, core] -> tile
    tile_core = np.empty(T, np.int64)
    tile_slot = np.empty(T, np.int64)
    tile_core[tile_at.reshape(-1)] = np.tile(np.arange(NCORES), TPC)
    tile_slot[tile_at.reshape(-1)] = np.repeat(np.arange(TPC), NCORES)

    nodes = np.arange(NP, dtype=np.int64)
    qbase, qrem = TPC // NCHUNK, TPC % NCHUNK
    qsizes = [qbase + (1 if j < qrem else 0) for j in range(NCHUNK)]
    qstart = np.cumsum([0] + qsizes)
    qof = np.concatenate([np.full(qsizes[j], j, np.int64)
                          for j in range(NCHUNK)]) if TPC else np.zeros(0, np.int64)
    nslot = tile_slot[nodes // P]
    ncore = tile_core[nodes // P]
    nq = qof[nslot]
    zchunk = nq
    zlocal = (ncore * np.array(qsizes)[nq] * P
              + (nslot - qstart[nq]) * P + nodes % P)

    hrow = []
    colnode = []
    for c in range(NCORES):
        own = list(tile_at[:, c])
        rest = [t for t in range(T) if tile_core[t] != c]
        sq = np.array(own + rest, dtype=np.int64)
        pos = np.empty(T, np.int64)
        pos[sq] = np.arange(T)
        hrow.append(pos[nodes // P] * P + nodes % P)
        colnode.append((sq[:, None] * P + np.arange(P)[None, :]).reshape(-1))

    ecore = tile_core[etile]
    eslot = tile_slot[etile]
    edloc = dst % P
    supers = [list(range(g, min(g + G, TPC))) for g in range(0, TPC, G)]

    def edge_lists(erow_per_core, echunk_per_core):
        lists = [[[None] * NCHUNK for _ in range(TPC)] for _ in range(NCORES)]
        for c in range(NCORES):
            m = ecore == c
            er = erow_per_core[c][m]
            es = eslot[m]
            ed = edloc[m]
            ech = echunk_per_core[c][m]
            key = np.lexsort((ed, ech, es))
            er, es, ed, ech = er[key], es[key], ed[key], ech[key]
            grp = es * NCHUNK + ech
            bounds = np.searchsorted(grp, np.arange(TPC * NCHUNK + 1))
            for s in range(TPC):
                for ch in range(NCHUNK):
                    a, b = bounds[s * NCHUNK + ch], bounds[s * NCHUNK + ch + 1]
                    lists[c][s][ch] = (er[a:b], ed[a:b])
        q16 = np.zeros((TPC, NCHUNK), np.int64)
        for s in range(TPC):
            for ch in range(NCHUNK):
                mx = max(len(lists[c][s][ch][0]) for c in range(NCORES))
                q16[s, ch] = mx
        return lists, q16

    l1, q1 = edge_lists([hrow[c][src] % CH for c in range(NCORES)],
                        [hrow[c][src] // CH for c in range(NCORES)])
    meta1, dl1, ix1 = _build_layer_schedule(l1, q1, supers, TPC)
    l2, q2 = edge_lists([zlocal[src] for _ in range(NCORES)],
                        [zchunk[src] for _ in range(NCORES)])
    meta2, dl2, ix2 = _build_layer_schedule(l2, q2, supers, TPC)

    plan = {
        "N": N, "F_IN": F_IN, "H": H, "C": C, "NP": NP, "T": T, "TPC": TPC,
        "CH": CH, "meta1": meta1, "meta2": meta2, "tile_at": tile_at,
        "qsizes": qsizes, "qstart": list(qstart), "qof": list(qof),
    }
    xpad = np.zeros((NP, F_IN), np.float32)
    xpad[:N] = np.asarray(x, np.float32)
    percore = []
    for c in range(NCORES):
        xT = np.ascontiguousarray(xpad[colnode[c]].T)
        dinvsrc = np.ascontiguousarray(dinv[colnode[c]].reshape(T, P).T)
        ownnodes = (tile_at[:, c][:, None] * P +
                    np.arange(P)[None, :]).reshape(-1)
        dinvdst = np.ascontiguousarray(dinv[ownnodes].reshape(TPC, P).T)
        percore.append({
            "xT": xT,
            "W1": np.asarray(W1, np.float32),
            "W2": np.asarray(W2, np.float32),
            "b1b": np.tile(np.asarray(b1, np.float32)[None, :], (P, 1)),
            "b2b": np.tile(np.asarray(b2, np.float32)[None, :], (P, 1)),
            "iota": np.tile(np.arange(P, dtype=np.float32)[None, :], (P, 1)),
            "dinvsrc": dinvsrc,
            "dinvdst": dinvdst,
            "dl1": dl1[c], "ix1": ix1[c],
            "dl2": dl2[c], "ix2": ix2[c],
        })
    plan["percore"] = percore
    return plan


def _build(plan):
    F_IN, H, C = plan["F_IN"], plan["H"], plan["C"]
    NP, T, TPC, CH = plan["NP"], plan["T"], plan["TPC"], plan["CH"]
    pc0 = plan["percore"][0]
    f32 = mybir.dt.float32

    nc = bacc.Bacc("TRN2", target_bir_lowering=False, debug=False,
                   num_devices=NCORES)
    t_xT = nc.dram_tensor("xT", [F_IN, NP], f32, kind="ExternalInput")
    t_W1 = nc.dram_tensor("W1", [F_IN, H], f32, kind="ExternalInput")
    t_W2 = nc.dram_tensor("W2", [H, C], f32, kind="ExternalInput")
    t_b1b = nc.dram_tensor("b1b", [P, H], f32, kind="ExternalInput")
    t_b2b = nc.dram_tensor("b2b", [P, C], f32, kind="ExternalInput")
    t_iota = nc.dram_tensor("iota", [P, P], f32, kind="ExternalInput")
    t_dsrc = nc.dram_tensor("dinvsrc", [P, T], f32, kind="ExternalInput")
    t_ddst = nc.dram_tensor("dinvdst", [P, TPC], f32, kind="ExternalInput")
    t_dl1 = nc.dram_tensor("dl1", list(pc0["dl1"].shape), f32,
                           kind="ExternalInput")
    t_ix1 = nc.dram_tensor("ix1", list(pc0["ix1"].shape), mybir.dt.int16,
                           kind="ExternalInput")
    t_dl2 = nc.dram_tensor("dl2", list(pc0["dl2"].shape), f32,
                           kind="ExternalInput")
    t_ix2 = nc.dram_tensor("ix2", list(pc0["ix2"].shape), mybir.dt.int16,
                           kind="ExternalInput")
    t_emb = nc.dram_tensor("emb", [TPC * P, H], f32, kind="ExternalOutput")
    t_logp = nc.dram_tensor("logp", [TPC * P, C], f32, kind="ExternalOutput")

    with tile.TileContext(nc) as tc:
        with (
            tc.tile_pool(name="const", bufs=1) as cp,
            tc.tile_pool(name="xin", bufs=2) as xp,
            tc.tile_pool(name="hst", bufs=3) as hp,
            tc.tile_pool(name="gbuf", bufs=2) as gp,
            tc.tile_pool(name="spool", bufs=3) as sp,
            tc.tile_pool(name="epool", bufs=4) as ep,
            tc.tile_pool(name="opool", bufs=3) as op,
            tc.tile_pool(name="psagg", bufs=G, space="PSUM") as pp,
            tc.tile_pool(name="dram", bufs=1, space="DRAM") as dp,
        ):
            def load_const(t, shape, nm, dtype=f32):
                s = cp.tile(shape, dtype, name=nm, tag=nm)
                nc.sync.dma_start(out=s[:], in_=t[:, :])
                return s

            W1s = load_const(t_W1, [F_IN, H], "cW1")
            W2s = load_const(t_W2, [H, C], "cW2")
            b1s = load_const(t_b1b, [P, H], "cb1")
            b2s = load_const(t_b2b, [P, C], "cb2")
            iot = load_const(t_iota, [P, P], "ciota")
            dsrc = load_const(t_dsrc, [P, T], "cdsrc")
            ddst = load_const(t_ddst, [P, TPC], "cddst")
            dl1s = load_const(t_dl1, list(pc0["dl1"].shape), "cdl1")
            ix1s = load_const(t_ix1, list(pc0["ix1"].shape), "cix1", mybir.dt.int16)
            dl2s = load_const(t_dl2, list(pc0["dl2"].shape), "cdl2")
            ix2s = load_const(t_ix2, list(pc0["ix2"].shape), "cix2", mybir.dt.int16)
            ident = cp.tile([P, P], f32)
            make_identity(nc, ident[:])

            htabs = [dp.tile([CH, H], f32, name=f"htab{i}", tag=f"htab{i}")
                     for i in range(NCHUNK)]
            qsizes, qstart = plan["qsizes"], plan["qstart"]
            qof = plan["qof"]
            zbs = [dp.tile([max(qsizes[j], 1) * P, C], f32, name=f"zb{j}",
                           tag=f"zb{j}") for j in range(NCHUNK)]
            ztabs = [dp.tile([max(qsizes[j], 1) * NCORES * P, C], f32,
                             name=f"ztab{j}", tag=f"ztab{j}")
                     for j in range(NCHUNK)]

            # ---- Phase A: h' table
            XCW = 28 * P if NP % (28 * P) == 0 else P
            ntc = XCW // P
            for x0 in range(0, NP, XCW):
                xc = xp.tile([F_IN, XCW], f32, tag="xc")
                nc.sync.dma_start(out=xc[:], in_=t_xT[:, x0 : x0 + XCW])
                for jj in range(0, ntc, XB):
                    nbh = min(XB, ntc - jj)
                    hs = hp.tile([P, XB * H], f32, tag="hs")
                    for k in range(nbh):
                        gt = (x0 // P) + jj + k
                        psA = pp.tile([P, H], f32, tag="agg")
                        nc.tensor.matmul(
                            out=psA[:],
                            lhsT=xc[:, (jj + k) * P : (jj + k + 1) * P],
                            rhs=W1s[:],
                            start=True, stop=True,
                        )
                        nc.vector.tensor_tensor(
                            out=hs[:, k * H : (k + 1) * H], in0=psA[:],
                            in1=dsrc[:, gt : gt + 1].to_broadcast([P, H]),
                            op=mybir.AluOpType.mult,
                        )
                    r0 = (x0 // P + jj) * P
                    chn, rloc = r0 // CH, r0 % CH
                    nc.sync.dma_start(
                        out=htabs[chn][rloc : rloc + nbh * P, :].rearrange(
                            "(a p) h -> p a h", p=P),
                        in_=hs[:, : nbh * H].rearrange("p (a h) -> p a h", h=H),
                    )

            def agg_layer(meta, dls, ixs, table, Fdim, epilogue,
                          after_slot=None):
                for sup in meta["supers"]:
                    nb = sup["nb"]
                    psums = {}
                    if nb:
                        gb = gp.tile([P, nb * Fdim], f32, tag="gb")
                        for call in sup["calls"]:
                            ch, Q = call["chunk"], call["Q"]
                            d0 = call["blk0"] * Fdim
                            nc.gpsimd.dma_gather(
                                gb[:, d0 : d0 + (Q // P) * Fdim].rearrange(
                                    "p (a h) -> p a h", h=Fdim),
                                table[ch][:, :] if isinstance(table, list)
                                else table[ch * CH : (ch + 1) * CH, :],
                                ixs[:, call["i16"] : call["i16"] + Q // 16],
                                Q, Q, Fdim,
                                single_packet=False,
                            )
                        sides = sup["sides"]
                        for i0 in range(0, len(sides), SG):
                            grp = sides[i0 : i0 + SG]
                            k = len(grp)
                            st = sp.tile([P, SG * P], f32, tag="s")
                            c0 = grp[0]["col"]
                            bca = dls[:, c0 : c0 + k].to_broadcast([P, k, P])
                            iap = iot[:, :]
                            iota_b = bass.AP(
                                iap.tensor, iap.offset,
                                [list(iap.ap[0]), [0, k], list(iap.ap[1])],
                            )
                            nc.vector.tensor_tensor(
                                out=st[:, : k * P].rearrange(
                                    "p (a q) -> p a q", q=P),
                                in0=bca, in1=iota_b,
                                op=mybir.AluOpType.is_equal,
                            )
                            for j, sd in enumerate(grp):
                                s = sd["slot"]
                                if s not in psums:
                                    psums[s] = pp.tile([P, Fdim], f32,
                                                       tag="agg", name="aggps")
                                nc.tensor.matmul(
                                    out=psums[s][:],
                                    lhsT=st[:, j * P : (j + 1) * P],
                                    rhs=gb[:, sd["b"] * Fdim :
                                           (sd["b"] + 1) * Fdim],
                                    start=sd["start"], stop=sd["stop"],
                                )
                                if sd["stop"]:
                                    epilogue(sd["slot"], psums[sd["slot"]])
                    for s in sup["slots"]:
                        if s not in psums:
                            psz = pp.tile([P, Fdim], f32, tag="agg")
                            nc.vector.memset(psz[:], 0.0)
                            epilogue(s, psz)
                    if after_slot is not None:
                        for s in sup["slots"]:
                            after_slot(s)

            def epi1(s, ps):
                ownt = op.tile([P, H], f32, tag="own1")
                nc.sync.dma_start(out=ownt[:],
                                  in_=htabs[0][s * P : (s + 1) * P, :])
                t1 = ep.tile([P, H], f32, tag="e1")
                nc.vector.tensor_add(out=t1[:], in0=ps[:], in1=ownt[:])
                t2 = ep.tile([P, H], f32, tag="e2")
                nc.vector.tensor_tensor(
                    out=t2[:], in0=t1[:],
                    in1=ddst[:, s : s + 1].to_broadcast([P, H]),
                    op=mybir.AluOpType.mult,
                )
                t3 = ep.tile([P, H], f32, tag="e3")
                nc.vector.tensor_add(out=t3[:], in0=t2[:], in1=b1s[:])
                h1 = ep.tile([P, H], f32, tag="h1")
                nc.scalar.activation(
                    out=h1[:], in_=t3[:],
                    func=mybir.ActivationFunctionType.Relu,
                )
                nc.sync.dma_start(out=t_emb[s * P : (s + 1) * P, :],
                                  in_=h1[:])
                pt = pp.tile([P, P], f32, tag="ptr", bufs=2)
                nc.tensor.transpose(out=pt[:], in_=h1[:], identity=ident[:])
                h1T = ep.tile([P, P], f32, tag="h1T")
                nc.vector.tensor_copy(out=h1T[:], in_=pt[:])
                pz = pp.tile([P, C], f32, tag="pz", bufs=2)
                nc.tensor.matmul(out=pz[:], lhsT=h1T[:], rhs=W2s[:],
                                 start=True, stop=True)
                zt = ep.tile([P, C], f32, tag="zt")
                nc.vector.tensor_tensor(
                    out=zt[:], in0=pz[:],
                    in1=ddst[:, s : s + 1].to_broadcast([P, C]),
                    op=mybir.AluOpType.mult,
                )
                j = qof[s]
                r = (s - qstart[j]) * P
                nc.sync.dma_start(out=zbs[j][r : r + P, :], in_=zt[:])

            def fire_ag(j):
                nc.gpsimd.collective_compute(
                    "AllGather",
                    mybir.AluOpType.bypass,
                    replica_groups=[list(range(NCORES))],
                    ins=[zbs[j].opt()],
                    outs=[ztabs[j].opt()],
                )

            def after_slot1(s):
                for j in range(NCHUNK):
                    if qsizes[j] and s == qstart[j] + qsizes[j] - 1:
                        fire_ag(j)

            agg_layer(plan["meta1"], dl1s, ix1s, htabs, H, epi1,
                      after_slot=after_slot1)

            def epi2(s, ps):
                zown = op.tile([P, C], f32, tag="own2")
                j = qof[s]
                r = (s - qstart[j]) * P
                nc.sync.dma_start(out=zown[:], in_=zbs[j][r : r + P, :])
                t1 = ep.tile([P, C], f32, tag="f1")
                nc.vector.tensor_add(out=t1[:], in0=ps[:], in1=zown[:])
                t2 = ep.tile([P, C], f32, tag="f2")
                nc.vector.tensor_tensor(
                    out=t2[:], in0=t1[:],
                    in1=ddst[:, s : s + 1].to_broadcast([P, C]),
                    op=mybir.AluOpType.mult,
                )
                t3 = ep.tile([P, C], f32, tag="f3")
                nc.vector.tensor_add(out=t3[:], in0=t2[:], in1=b2s[:])
                mx = ep.tile([P, 1], f32, tag="mx")
                nc.vector.reduce_max(out=mx[:], in_=t3[:],
                                     axis=mybir.AxisListType.X)
                xcc = ep.tile([P, C], f32, tag="xcc")
                nc.vector.tensor_tensor(
                    out=xcc[:], in0=t3[:], in1=mx[:].to_broadcast([P, C]),
                    op=mybir.AluOpType.subtract,
                )
                exv = ep.tile([P, C], f32, tag="exv")
                smv = ep.tile([P, 1], f32, tag="smv")
                nc.scalar.activation(
                    out=exv[:], in_=xcc[:],
                    func=mybir.ActivationFunctionType.Exp,
                    accum_out=smv[:],
                )
                lsv = ep.tile([P, 1], f32, tag="lsv")
                nc.scalar.activation(
                    out=lsv[:], in_=smv[:],
                    func=mybir.ActivationFunctionType.Ln,
                )
                ov = ep.tile([P, C], f32, tag="ov")
                nc.vector.tensor_tensor(
                    out=ov[:], in0=xcc[:], in1=lsv[:].to_broadcast([P, C]),
                    op=mybir.AluOpType.subtract,
                )
                nc.sync.dma_start(out=t_logp[s * P : (s + 1) * P, :],
                                  in_=ov[:])

            agg_layer(plan["meta2"], dl2s, ix2s, ztabs, C, epi2)

    nc.compile()
    return nc


def _in_maps(plan):
    maps = []
    for c in range(NCORES):
        pc = plan["percore"][c]
        maps.append({
            "xT": pc["xT"], "W1": pc["W1"], "W2": pc["W2"],
            "b1b": pc["b1b"], "b2b": pc["b2b"], "iota": pc["iota"],
            "dinvsrc": pc["dinvsrc"], "dinvdst": pc["dinvdst"],
            "dl1": pc["dl1"], "ix1": pc["ix1"],
            "dl2": pc["dl2"], "ix2": pc["ix2"],
        })
    return maps


def _assemble(plan, results):
    N, H, C, TPC = plan["N"], plan["H"], plan["C"], plan["TPC"]
    NP = plan["NP"]
    tile_at = plan["tile_at"]
    emb = np.zeros((NP, H), np.float32)
    logp = np.zeros((NP, C), np.float32)
    for c in range(NCORES):
        e = results[c]["emb"]
        l = results[c]["logp"]
        for s in range(TPC):
            gt = tile_at[s, c]
            emb[gt * P : (gt + 1) * P] = e[s * P : (s + 1) * P]
            logp[gt * P : (gt + 1) * P] = l[s * P : (s + 1) * P]
    return logp[:N], emb[:N]


def kernel(x, W1, b1, W2, b2, edge_index, _trace=False, _want_time=False):
    plan = _plan(np.asarray(x), np.asarray(W1), np.asarray(b1),
                 np.asarray(W2), np.asarray(b2), np.asarray(edge_index))
    nc = _build(plan)
    res = run_bass_kernel_spmd(nc, _in_maps(plan),
                               core_ids=list(range(NCORES)), trace=_trace)
    logp, emb = _assemble(plan, res.results)
    if _want_time:
        return (logp, emb), res.exec_time_ns
    return logp, emb


# revision 9
# speedup vs baseline: 1.1069x; 1.0654x over previous
"""GCN (2-layer, symmetric-normalized adjacency) on 8 TRN2 NeuronCores.

Strategy:
  - Full f32. Host does graph preprocessing only (normalization constants,
    partitioning, index tables); all FLOPs on x/W run on device.
  - Nodes padded to NP; dst tiles of 128 nodes; tiles assigned to cores with
    count balancing (sort by edge count, deal round-robin).
  - GEMM1 per-core computes h' = dinv[src]*(x@W1) for ALL nodes into a DRAM
    table (per-core column order: own tiles first, so self-loop rows sit at
    static offsets).
  - Edge messages gathered from the table with bulk dma_gather (int16
    indices -> 4 src-row chunks; per-(tile,chunk) quotas padded to the max
    across cores so the program is SPMD-uniform).
  - Aggregation = one-hot matmuls on the TensorEngine: S[p,j] =
    (dstloc[p]==j), PSUM-accumulated per dst tile; self-loop row added from
    the table; epilogue relu(dinv*(agg+own)+b1).
  - z' = dinv*(h1@W2) shards AllGathered into a z table; layer 2 repeats the
    gather/aggregate structure and ends in log_softmax.
"""

import sys
import types

import numpy as np


def _install_ntff_hook():
    if "antenv.axon_hooks" in sys.modules:
        return
    try:
        from trn_agent_boot.trn_boot import _ntff_profile_via_ctypes

        hook = _ntff_profile_via_ctypes("/opt/axon/libaxon_pjrt.so")
    except Exception:
        hook = None
    mod = types.ModuleType("antenv.axon_hooks")
    mod.get_axon_ntff_profile_hook = lambda: hook
    mod.set_axon_ntff_profile_hook = lambda h: None
    sys.modules["antenv.axon_hooks"] = mod


_install_ntff_hook()

import concourse.bass as bass
import concourse.bacc as bacc
import concourse.tile as tile
import concourse.mybir as mybir
from concourse.bass_utils import run_bass_kernel_spmd
from concourse.masks import make_identity

P = 128
NCORES = 8
NCHUNK = 4
G = 6        # dst tiles per superbatch
SG = 16      # one-hot columns per S tile
XB = 4       # GEMM1 tiles per staging/DMA batch
INVALID = 1000.0


def _round_up(x, m):
    return -(-x // m) * m


def _build_layer_schedule(lists, q16, supers, TPC):
    """Static (core-uniform) schedule for one layer + per-core dstloc/idx.

    lists[c][s][ch] = (rows_in_chunk, dst_local) per core/slot/chunk,
    q16[s, ch] = padded (max-over-cores) count, multiple of 16.
    """
    meta = {"supers": []}
    dl_cols = []     # column fill instructions: (col, seg info)
    col_count = 0
    idx16_count = 0
    idxw = [[] for _ in range(NCORES)]

    for slots in supers:
        calls = []
        segs = []    # (slot, start_pos, qlen, chunk)
        pos = 0
        for ch in range(NCHUNK):
            q_sum = int(sum(q16[s, ch] for s in slots))
            if q_sum == 0:
                continue
            Q = _round_up(q_sum, P)
            calls.append({"chunk": ch, "Q": Q, "i16": idx16_count,
                          "blk0": pos // P})
            for c in range(NCORES):
                arr = np.zeros(Q, np.int64)
                o = 0
                for s in slots:
                    q = int(q16[s, ch])
                    if q == 0:
                        continue
                    rows = lists[c][s][ch][0]
                    arr[o : o + len(rows)] = rows
                    o += q
                w = arr.astype(np.int16).reshape(Q // 16, 16).T
                idxw[c].append(np.tile(w, (NCORES, 1)))
            for s in slots:
                q = int(q16[s, ch])
                if q:
                    segs.append((s, pos, q, ch))
                    pos += q
            pos += Q - q_sum
            idx16_count += Q // 16
        M = pos
        assert M % P == 0
        nb = M // P
        sides = []
        slot_sides = {}
        for b in range(nb):
            lo, hi = b * P, (b + 1) * P
            for (s, sp, ln, ch) in segs:
                if sp < hi and sp + ln > lo:
                    sd = {"b": b, "col": col_count, "slot": s,
                          "sp": sp, "ln": ln, "chunk": ch}
                    sides.append(sd)
                    slot_sides.setdefault(s, []).append(sd)
                    col_count += 1
        for s, sl in slot_sides.items():
            for sd in sl:
                sd["start"] = sd is sl[0]
                sd["stop"] = sd is sl[-1]
        meta["supers"].append({"slots": slots, "calls": calls, "nb": nb,
                               "sides": sides})
    meta["ncols"] = max(col_count, 1)
    meta["nidx16"] = max(idx16_count, 1)

    dl = [np.full((P, meta["ncols"]), INVALID, np.float32)
          for _ in range(NCORES)]
    for sup in meta["supers"]:
        for sd in sup["sides"]:
            sp, ln, s, ch = sd["sp"], sd["ln"], sd["slot"], sd["chunk"]
            lo, hi = sd["b"] * P, (sd["b"] + 1) * P
            a = max(sp, lo)
            b_ = min(sp + ln, hi)
            r0 = a - sp
            for c in range(NCORES):
                ed = lists[c][s][ch][1]
                r1 = min(b_ - sp, len(ed))
                if r1 > r0:
                    pidx = (a - lo) + np.arange(r1 - r0)
                    dl[c][pidx, sd["col"]] = ed[r0:r1]

    idxcat = []
    for c in range(NCORES):
        if idxw[c]:
            idxcat.append(np.ascontiguousarray(
                np.concatenate(idxw[c], axis=1)))
        else:
            idxcat.append(np.zeros((P, 1), np.int16))
    return meta, dl, idxcat


def _plan(x, W1, b1, W2, b2, edge_index):
    N, F_IN = x.shape
    H = W1.shape[1]
    C = W2.shape[1]
    src = np.asarray(edge_index[0], dtype=np.int64)
    dst = np.asarray(edge_index[1], dtype=np.int64)

    NP = _round_up(N, P * NCORES)
    T = NP // P
    TPC = T // NCORES
    CH = NP // NCHUNK
    assert CH <= 32767 and CH % P == 0

    deg = np.bincount(dst, minlength=NP).astype(np.float64) + 1.0
    dinv = (1.0 / np.sqrt(deg)).astype(np.float32)
    dinv[N:] = 0.0

    etile = dst // P
    counts = np.bincount(etile, minlength=T)
    order = np.argsort(-counts, kind="stable")
    tile_at = order.reshape(TPC, NCORES)            # [slot, core] -> tile
    tile_core = np.empty(T, np.int64)
    tile_slot = np.empty(T, np.int64)
    tile_core[tile_at.reshape(-1)] = np.tile(np.arange(NCORES), TPC)
    tile_slot[tile_at.reshape(-1)] = np.repeat(np.arange(TPC), NCORES)

    nodes = np.arange(NP, dtype=np.int64)
    qbase, qrem = TPC // NCHUNK, TPC % NCHUNK
    qsizes = [qbase + (1 if j < qrem else 0) for j in range(NCHUNK)]
    qstart = np.cumsum([0] + qsizes)
    qof = np.concatenate([np.full(qsizes[j], j, np.int64)
                          for j in range(NCHUNK)]) if TPC else np.zeros(0, np.int64)
    nslot = tile_slot[nodes // P]
    ncore = tile_core[nodes // P]
    nq = qof[nslot]
    zchunk = nq
    zlocal = (ncore * np.array(qsizes)[nq] * P
              + (nslot - qstart[nq]) * P + nodes % P)

    hrow = []
    colnode = []
    for c in range(NCORES):
        own = list(tile_at[:, c])
        rest = [t for t in range(T) if tile_core[t] != c]
        sq = np.array(own + rest, dtype=np.int64)
        pos = np.empty(T, np.int64)
        pos[sq] = np.arange(T)
        hrow.append(pos[nodes // P] * P + nodes % P)
        colnode.append((sq[:, None] * P + np.arange(P)[None, :]).reshape(-1))

    ecore = tile_core[etile]
    eslot = tile_slot[etile]
    edloc = dst % P
    supers = [list(range(g, min(g + G, TPC))) for g in range(0, TPC, G)]

    def edge_lists(erow_per_core, echunk_per_core):
        lists = [[[None] * NCHUNK for _ in range(TPC)] for _ in range(NCORES)]
        for c in range(NCORES):
            m = ecore == c
            er = erow_per_core[c][m]
            es = eslot[m]
            ed = edloc[m]
            ech = echunk_per_core[c][m]
            key = np.lexsort((ed, ech, es))
            er, es, ed, ech = er[key], es[key], ed[key], ech[key]
            grp = es * NCHUNK + ech
            bounds = np.searchsorted(grp, np.arange(TPC * NCHUNK + 1))
            for s in range(TPC):
                for ch in range(NCHUNK):
                    a, b = bounds[s * NCHUNK + ch], bounds[s * NCHUNK + ch + 1]
                    lists[c][s][ch] = (er[a:b], ed[a:b])
        q16 = np.zeros((TPC, NCHUNK), np.int64)
        for s in range(TPC):
            for ch in range(NCHUNK):
                mx = max(len(lists[c][s][ch][0]) for c in range(NCORES))
                q16[s, ch] = mx
        return lists, q16

    l1, q1 = edge_lists([hrow[c][src] % CH for c in range(NCORES)],
                        [hrow[c][src] // CH for c in range(NCORES)])
    meta1, dl1, ix1 = _build_layer_schedule(l1, q1, supers, TPC)
    l2, q2 = edge_lists([zlocal[src] for _ in range(NCORES)],
                        [zchunk[src] for _ in range(NCORES)])
    meta2, dl2, ix2 = _build_layer_schedule(l2, q2, supers, TPC)

    plan = {
        "N": N, "F_IN": F_IN, "H": H, "C": C, "NP": NP, "T": T, "TPC": TPC,
        "CH": CH, "meta1": meta1, "meta2": meta2, "tile_at": tile_at,
        "qsizes": qsizes, "qstart": list(qstart), "qof": list(qof),
    }
    xpad = np.zeros((NP, F_IN), np.float32)
    xpad[:N] = np.asarray(x, np.float32)
    percore = []
    for c in range(NCORES):
        xT = np.ascontiguousarray(xpad[colnode[c]].T)
        dinvsrc = np.ascontiguousarray(dinv[colnode[c]].reshape(T, P).T)
        ownnodes = (tile_at[:, c][:, None] * P +
                    np.arange(P)[None, :]).reshape(-1)
        dinvdst = np.ascontiguousarray(dinv[ownnodes].reshape(TPC, P).T)
        percore.append({
            "xT": xT,
            "W1": np.asarray(W1, np.float32),
            "W2": np.asarray(W2, np.float32),
            "b1b": np.tile(np.asarray(b1, np.float32)[None, :], (P, 1)),
            "b2b": np.tile(np.asarray(b2, np.float32)[None, :], (P, 1)),
            "iota": np.tile(np.arange(P, dtype=np.float32)[None, :], (P, 1)),
            "dinvsrc": dinvsrc,
            "dinvdst": dinvdst,
            "dl1": dl1[c], "ix1": ix1[c],
            "dl2": dl2[c], "ix2": ix2[c],
        })
    plan["percore"] = percore
    return plan


def _build(plan):
    F_IN, H, C = plan["F_IN"], plan["H"], plan["C"]
    NP, T, TPC, CH = plan["NP"], plan["T"], plan["TPC"], plan["CH"]
    pc0 = plan["percore"][0]
    f32 = mybir.dt.float32

    nc = bacc.Bacc("TRN2", target_bir_lowering=False, debug=False,
                   num_devices=NCORES)
    t_xT = nc.dram_tensor("xT", [F_IN, NP], f32, kind="ExternalInput")
    t_W1 = nc.dram_tensor("W1", [F_IN, H], f32, kind="ExternalInput")
    t_W2 = nc.dram_tensor("W2", [H, C], f32, kind="ExternalInput")
    t_b1b = nc.dram_tensor("b1b", [P, H], f32, kind="ExternalInput")
    t_b2b = nc.dram_tensor("b2b", [P, C], f32, kind="ExternalInput")
    t_iota = nc.dram_tensor("iota", [P, P], f32, kind="ExternalInput")
    t_dsrc = nc.dram_tensor("dinvsrc", [P, T], f32, kind="ExternalInput")
    t_ddst = nc.dram_tensor("dinvdst", [P, TPC], f32, kind="ExternalInput")
    t_dl1 = nc.dram_tensor("dl1", list(pc0["dl1"].shape), f32,
                           kind="ExternalInput")
    t_ix1 = nc.dram_tensor("ix1", list(pc0["ix1"].shape), mybir.dt.int16,
                           kind="ExternalInput")
    t_dl2 = nc.dram_tensor("dl2", list(pc0["dl2"].shape), f32,
                           kind="ExternalInput")
    t_ix2 = nc.dram_tensor("ix2", list(pc0["ix2"].shape), mybir.dt.int16,
                           kind="ExternalInput")
    t_emb = nc.dram_tensor("emb", [TPC * P, H], f32, kind="ExternalOutput")
    t_logp = nc.dram_tensor("logp", [TPC * P, C], f32, kind="ExternalOutput")

    with tile.TileContext(nc) as tc:
        with (
            tc.tile_pool(name="const", bufs=1) as cp,
            tc.tile_pool(name="xin", bufs=2) as xp,
            tc.tile_pool(name="hst", bufs=3) as hp,
            tc.tile_pool(name="gbuf", bufs=2) as gp,
            tc.tile_pool(name="spool", bufs=3) as sp,
            tc.tile_pool(name="epool", bufs=4) as ep,
            tc.tile_pool(name="opool", bufs=3) as op,
            tc.tile_pool(name="psagg", bufs=8, space="PSUM") as pp,
            tc.tile_pool(name="dram", bufs=1, space="DRAM") as dp,
        ):
            def load_const(t, shape, nm, dtype=f32):
                s = cp.tile(shape, dtype, name=nm, tag=nm)
                nc.sync.dma_start(out=s[:], in_=t[:, :])
                return s

            W1s = load_const(t_W1, [F_IN, H], "cW1")
            W2s = load_const(t_W2, [H, C], "cW2")
            b1s = load_const(t_b1b, [P, H], "cb1")
            b2s = load_const(t_b2b, [P, C], "cb2")
            iot = load_const(t_iota, [P, P], "ciota")
            dsrc = load_const(t_dsrc, [P, T], "cdsrc")
            ddst = load_const(t_ddst, [P, TPC], "cddst")
            dl1s = load_const(t_dl1, list(pc0["dl1"].shape), "cdl1")
            ix1s = load_const(t_ix1, list(pc0["ix1"].shape), "cix1", mybir.dt.int16)
            dl2s = load_const(t_dl2, list(pc0["dl2"].shape), "cdl2")
            ix2s = load_const(t_ix2, list(pc0["ix2"].shape), "cix2", mybir.dt.int16)
            ident = cp.tile([P, P], f32)
            make_identity(nc, ident[:])

            htabs = [dp.tile([CH, H], f32, name=f"htab{i}", tag=f"htab{i}")
                     for i in range(NCHUNK)]
            qsizes, qstart = plan["qsizes"], plan["qstart"]
            qof = plan["qof"]
            zbs = [dp.tile([max(qsizes[j], 1) * P, C], f32, name=f"zb{j}",
                           tag=f"zb{j}") for j in range(NCHUNK)]
            ztabs = [dp.tile([max(qsizes[j], 1) * NCORES * P, C], f32,
                             name=f"ztab{j}", tag=f"ztab{j}")
                     for j in range(NCHUNK)]

            # ---- Phase A: h' table
            XCW = 28 * P if NP % (28 * P) == 0 else P
            ntc = XCW // P
            for x0 in range(0, NP, XCW):
                xc = xp.tile([F_IN, XCW], f32, tag="xc")
                nc.sync.dma_start(out=xc[:], in_=t_xT[:, x0 : x0 + XCW])
                for jj in range(0, ntc, XB):
                    nbh = min(XB, ntc - jj)
                    hs = hp.tile([P, XB * H], f32, tag="hs")
                    for k in range(nbh):
                        gt = (x0 // P) + jj + k
                        psA = pp.tile([P, H], f32, tag="agg")
                        nc.tensor.matmul(
                            out=psA[:],
                            lhsT=xc[:, (jj + k) * P : (jj + k + 1) * P],
                            rhs=W1s[:],
                            start=True, stop=True,
                        )
                        if gt % 2 == 0:
                            nc.vector.tensor_tensor(
                                out=hs[:, k * H : (k + 1) * H], in0=psA[:],
                                in1=dsrc[:, gt : gt + 1].to_broadcast([P, H]),
                                op=mybir.AluOpType.mult,
                            )
                        else:
                            nc.scalar.activation(
                                out=hs[:, k * H : (k + 1) * H], in_=psA[:],
                                func=mybir.ActivationFunctionType.Copy,
                                scale=dsrc[:, gt : gt + 1],
                            )
                    r0 = (x0 // P + jj) * P
                    chn, rloc = r0 // CH, r0 % CH
                    nc.sync.dma_start(
                        out=htabs[chn][rloc : rloc + nbh * P, :].rearrange(
                            "(a p) h -> p a h", p=P),
                        in_=hs[:, : nbh * H].rearrange("p (a h) -> p a h", h=H),
                    )

            def agg_layer(meta, dls, ixs, table, Fdim, epilogue,
                          after_slot=None):
                for sup in meta["supers"]:
                    nb = sup["nb"]
                    psums = {}
                    if nb:
                        gb = gp.tile([P, nb * Fdim], f32, tag="gb")
                        for call in sup["calls"]:
                            ch, Q = call["chunk"], call["Q"]
                            d0 = call["blk0"] * Fdim
                            nc.gpsimd.dma_gather(
                                gb[:, d0 : d0 + (Q // P) * Fdim].rearrange(
                                    "p (a h) -> p a h", h=Fdim),
                                table[ch][:, :] if isinstance(table, list)
                                else table[ch * CH : (ch + 1) * CH, :],
                                ixs[:, call["i16"] : call["i16"] + Q // 16],
                                Q, Q, Fdim,
                                single_packet=False,
                            )
                        sides = sup["sides"]
                        for i0 in range(0, len(sides), SG):
                            grp = sides[i0 : i0 + SG]
                            k = len(grp)
                            st = sp.tile([P, SG * P], f32, tag="s")
                            c0 = grp[0]["col"]
                            bca = dls[:, c0 : c0 + k].to_broadcast([P, k, P])
                            iap = iot[:, :]
                            iota_b = bass.AP(
                                iap.tensor, iap.offset,
                                [list(iap.ap[0]), [0, k], list(iap.ap[1])],
                            )
                            nc.vector.tensor_tensor(
                                out=st[:, : k * P].rearrange(
                                    "p (a q) -> p a q", q=P),
                                in0=bca, in1=iota_b,
                                op=mybir.AluOpType.is_equal,
                            )
                            for j, sd in enumerate(grp):
                                s = sd["slot"]
                                if s not in psums:
                                    psums[s] = pp.tile([P, Fdim], f32,
                                                       tag="agg", name="aggps")
                                nc.tensor.matmul(
                                    out=psums[s][:],
                                    lhsT=st[:, j * P : (j + 1) * P],
                                    rhs=gb[:, sd["b"] * Fdim :
                                           (sd["b"] + 1) * Fdim],
                                    start=sd["start"], stop=sd["stop"],
                                )
                                if sd["stop"]:
                                    epilogue(sd["slot"], psums[sd["slot"]])
                    for s in sup["slots"]:
                        if s not in psums:
                            psz = pp.tile([P, Fdim], f32, tag="agg")
                            nc.vector.memset(psz[:], 0.0)
                            epilogue(s, psz)
                    if after_slot is not None:
                        for s in sup["slots"]:
                            after_slot(s)

            def epi1(s, ps):
                ownt = op.tile([P, H], f32, tag="own1")
                nc.sync.dma_start(out=ownt[:],
                                  in_=htabs[0][s * P : (s + 1) * P, :])
                t1 = ep.tile([P, H], f32, tag="e1")
                nc.vector.tensor_add(out=t1[:], in0=ps[:], in1=ownt[:])
                t2 = ep.tile([P, H], f32, tag="e2")
                nc.vector.tensor_tensor(
                    out=t2[:], in0=t1[:],
                    in1=ddst[:, s : s + 1].to_broadcast([P, H]),
                    op=mybir.AluOpType.mult,
                )
                t3 = ep.tile([P, H], f32, tag="e3")
                nc.vector.tensor_add(out=t3[:], in0=t2[:], in1=b1s[:])
                h1 = ep.tile([P, H], f32, tag="h1")
                nc.scalar.activation(
                    out=h1[:], in_=t3[:],
                    func=mybir.ActivationFunctionType.Relu,
                )
                nc.sync.dma_start(out=t_emb[s * P : (s + 1) * P, :],
                                  in_=h1[:])
                pt = pp.tile([P, P], f32, tag="agg")
                nc.tensor.transpose(out=pt[:], in_=h1[:], identity=ident[:])
                h1T = ep.tile([P, P], f32, tag="h1T")
                nc.vector.tensor_copy(out=h1T[:], in_=pt[:])
                pz = pp.tile([P, P], f32, tag="agg", name="pzt")
                nc.tensor.matmul(out=pz[:, :C], lhsT=h1T[:], rhs=W2s[:],
                                 start=True, stop=True)
                zt = ep.tile([P, C], f32, tag="zt")
                nc.vector.tensor_tensor(
                    out=zt[:], in0=pz[:, :C],
                    in1=ddst[:, s : s + 1].to_broadcast([P, C]),
                    op=mybir.AluOpType.mult,
                )
                j = qof[s]
                r = (s - qstart[j]) * P
                nc.sync.dma_start(out=zbs[j][r : r + P, :], in_=zt[:])

            def fire_ag(j):
                nc.gpsimd.collective_compute(
                    "AllGather",
                    mybir.AluOpType.bypass,
                    replica_groups=[list(range(NCORES))],
                    ins=[zbs[j].opt()],
                    outs=[ztabs[j].opt()],
                )

            def after_slot1(s):
                for j in range(NCHUNK):
                    if qsizes[j] and s == qstart[j] + qsizes[j] - 1:
                        fire_ag(j)

            agg_layer(plan["meta1"], dl1s, ix1s, htabs, H, epi1,
                      after_slot=after_slot1)

            def epi2(s, ps):
                zown = op.tile([P, C], f32, tag="own2")
                j = qof[s]
                r = (s - qstart[j]) * P
                nc.sync.dma_start(out=zown[:], in_=zbs[j][r : r + P, :])
                t1 = ep.tile([P, C], f32, tag="f1")
                nc.vector.tensor_add(out=t1[:], in0=ps[:], in1=zown[:])
                t2 = ep.tile([P, C], f32, tag="f2")
                nc.vector.tensor_tensor(
                    out=t2[:], in0=t1[:],
                    in1=ddst[:, s : s + 1].to_broadcast([P, C]),
                    op=mybir.AluOpType.mult,
                )
                t3 = ep.tile([P, C], f32, tag="f3")
                nc.vector.tensor_add(out=t3[:], in0=t2[:], in1=b2s[:])
                mx = ep.tile([P, 1], f32, tag="mx")
                nc.vector.reduce_max(out=mx[:], in_=t3[:],
                                     axis=mybir.AxisListType.X)
                xcc = ep.tile([P, C], f32, tag="xcc")
                nc.vector.tensor_tensor(
                    out=xcc[:], in0=t3[:], in1=mx[:].to_broadcast([P, C]),
                    op=mybir.AluOpType.subtract,
                )
                exv = ep.tile([P, C], f32, tag="exv")
                smv = ep.tile([P, 1], f32, tag="smv")
                nc.scalar.activation(
                    out=exv[:], in_=xcc[:],
                    func=mybir.ActivationFunctionType.Exp,
                    accum_out=smv[:],
                )
                lsv = ep.tile([P, 1], f32, tag="lsv")
                nc.scalar.activation(
                    out=lsv[:], in_=smv[:],
                    func=mybir.ActivationFunctionType.Ln,
                )
                ov = ep.tile([P, C], f32, tag="ov")
                nc.vector.tensor_tensor(
                    out=ov[:], in0=xcc[:], in1=lsv[:].to_broadcast([P, C]),
                    op=mybir.AluOpType.subtract,
                )
                nc.sync.dma_start(out=t_logp[s * P : (s + 1) * P, :],
                                  in_=ov[:])

            agg_layer(plan["meta2"], dl2s, ix2s, ztabs, C, epi2)

    nc.compile()
    return nc


def _in_maps(plan):
    maps = []
    for c in range(NCORES):
        pc = plan["percore"][c]
        maps.append({
            "xT": pc["xT"], "W1": pc["W1"], "W2": pc["W2"],
            "b1b": pc["b1b"], "b2b": pc["b2b"], "iota": pc["iota"],
            "dinvsrc": pc["dinvsrc"], "dinvdst": pc["dinvdst"],
            "dl1": pc["dl1"], "ix1": pc["ix1"],
            "dl2": pc["dl2"], "ix2": pc["ix2"],
        })
    return maps


def _assemble(plan, results):
    N, H, C, TPC = plan["N"], plan["H"], plan["C"], plan["TPC"]
    NP = plan["NP"]
    tile_at = plan["tile_at"]
    emb = np.zeros((NP, H), np.float32)
    logp = np.zeros((NP, C), np.float32)
    for c in range(NCORES):
        e = results[c]["emb"]
        l = results[c]["logp"]
        for s in range(TPC):
            gt = tile_at[s, c]
            emb[gt * P : (gt + 1) * P] = e[s * P : (s + 1) * P]
            logp[gt * P : (gt + 1) * P] = l[s * P : (s + 1) * P]
    return logp[:N], emb[:N]


def kernel(x, W1, b1, W2, b2, edge_index, _trace=False, _want_time=False):
    plan = _plan(np.asarray(x), np.asarray(W1), np.asarray(b1),
                 np.asarray(W2), np.asarray(b2), np.asarray(edge_index))
    nc = _build(plan)
    res = run_bass_kernel_spmd(nc, _in_maps(plan),
                               core_ids=list(range(NCORES)), trace=_trace)
    logp, emb = _assemble(plan, res.results)
    if _want_time:
        return (logp, emb), res.exec_time_ns
    return logp, emb


# revision 12
# speedup vs baseline: 1.1121x; 1.0047x over previous
"""GCN (2-layer, symmetric-normalized adjacency) on 8 TRN2 NeuronCores.

Strategy:
  - Full f32. Host does graph preprocessing only (normalization constants,
    partitioning, index tables); all FLOPs on x/W run on device.
  - Nodes padded to NP; dst tiles of 128 nodes; tiles assigned to cores with
    count balancing (sort by edge count, deal round-robin).
  - GEMM1 per-core computes h' = dinv[src]*(x@W1) for ALL nodes into a DRAM
    table (per-core column order: own tiles first, so self-loop rows sit at
    static offsets).
  - Edge messages gathered from the table with bulk dma_gather (int16
    indices -> 4 src-row chunks; per-(tile,chunk) quotas padded to the max
    across cores so the program is SPMD-uniform).
  - Aggregation = one-hot matmuls on the TensorEngine: S[p,j] =
    (dstloc[p]==j), PSUM-accumulated per dst tile; self-loop row added from
    the table; epilogue relu(dinv*(agg+own)+b1).
  - z' = dinv*(h1@W2) shards AllGathered into a z table; layer 2 repeats the
    gather/aggregate structure and ends in log_softmax.
"""

import sys
import types

import numpy as np


def _install_ntff_hook():
    if "antenv.axon_hooks" in sys.modules:
        return
    try:
        from trn_agent_boot.trn_boot import _ntff_profile_via_ctypes

        hook = _ntff_profile_via_ctypes("/opt/axon/libaxon_pjrt.so")
    except Exception:
        hook = None
    mod = types.ModuleType("antenv.axon_hooks")
    mod.get_axon_ntff_profile_hook = lambda: hook
    mod.set_axon_ntff_profile_hook = lambda h: None
    sys.modules["antenv.axon_hooks"] = mod


_install_ntff_hook()

import concourse.bass as bass
import concourse.bacc as bacc
import concourse.tile as tile
import concourse.mybir as mybir
from concourse.bass_utils import run_bass_kernel_spmd
from concourse.masks import make_identity

P = 128
NCORES = 8
NCHUNK = 4
G = 6        # dst tiles per superbatch
SG = 16      # one-hot columns per S tile
XB = 4       # GEMM1 tiles per staging/DMA batch
INVALID = 1000.0


def _round_up(x, m):
    return -(-x // m) * m


def _build_layer_schedule(lists, q16, supers, TPC):
    """Static (core-uniform) schedule for one layer + per-core dstloc/idx.

    lists[c][s][ch] = (rows_in_chunk, dst_local) per core/slot/chunk,
    q16[s, ch] = padded (max-over-cores) count, multiple of 16.
    """
    meta = {"supers": []}
    dl_cols = []     # column fill instructions: (col, seg info)
    col_count = 0
    idx16_count = 0
    idxw = [[] for _ in range(NCORES)]

    for slots in supers:
        calls = []
        segs = []    # (slot, start_pos, qlen, chunk)
        pos = 0
        for ch in range(NCHUNK):
            q_sum = int(sum(q16[s, ch] for s in slots))
            if q_sum == 0:
                continue
            Q = _round_up(q_sum, P)
            calls.append({"chunk": ch, "Q": Q, "i16": idx16_count,
                          "blk0": pos // P})
            for c in range(NCORES):
                arr = np.zeros(Q, np.int64)
                o = 0
                for s in slots:
                    q = int(q16[s, ch])
                    if q == 0:
                        continue
                    rows = lists[c][s][ch][0]
                    arr[o : o + len(rows)] = rows
                    o += q
                w = arr.astype(np.int16).reshape(Q // 16, 16).T
                idxw[c].append(np.tile(w, (NCORES, 1)))
            for s in slots:
                q = int(q16[s, ch])
                if q:
                    segs.append((s, pos, q, ch))
                    pos += q
            pos += Q - q_sum
            idx16_count += Q // 16
        M = pos
        assert M % P == 0
        nb = M // P
        sides = []
        slot_sides = {}
        for b in range(nb):
            lo, hi = b * P, (b + 1) * P
            for (s, sp, ln, ch) in segs:
                if sp < hi and sp + ln > lo:
                    sd = {"b": b, "col": col_count, "slot": s,
                          "sp": sp, "ln": ln, "chunk": ch}
                    sides.append(sd)
                    slot_sides.setdefault(s, []).append(sd)
                    col_count += 1
        for s, sl in slot_sides.items():
            for sd in sl:
                sd["start"] = sd is sl[0]
                sd["stop"] = sd is sl[-1]
        meta["supers"].append({"slots": slots, "calls": calls, "nb": nb,
                               "sides": sides})
    meta["ncols"] = max(col_count, 1)
    meta["nidx16"] = max(idx16_count, 1)

    dl = [np.full((P, meta["ncols"]), INVALID, np.float32)
          for _ in range(NCORES)]
    for sup in meta["supers"]:
        for sd in sup["sides"]:
            sp, ln, s, ch = sd["sp"], sd["ln"], sd["slot"], sd["chunk"]
            lo, hi = sd["b"] * P, (sd["b"] + 1) * P
            a = max(sp, lo)
            b_ = min(sp + ln, hi)
            r0 = a - sp
            for c in range(NCORES):
                ed = lists[c][s][ch][1]
                r1 = min(b_ - sp, len(ed))
                if r1 > r0:
                    pidx = (a - lo) + np.arange(r1 - r0)
                    dl[c][pidx, sd["col"]] = ed[r0:r1]

    idxcat = []
    for c in range(NCORES):
        if idxw[c]:
            idxcat.append(np.ascontiguousarray(
                np.concatenate(idxw[c], axis=1)))
        else:
            idxcat.append(np.zeros((P, 1), np.int16))
    return meta, dl, idxcat


def _plan(x, W1, b1, W2, b2, edge_index):
    N, F_IN = x.shape
    H = W1.shape[1]
    C = W2.shape[1]
    src = np.asarray(edge_index[0], dtype=np.int64)
    dst = np.asarray(edge_index[1], dtype=np.int64)

    NP = _round_up(N, P * NCORES)
    T = NP // P
    TPC = T // NCORES
    CH = NP // NCHUNK
    assert CH <= 32767 and CH % P == 0

    deg = np.bincount(dst, minlength=NP).astype(np.float64) + 1.0
    dinv = (1.0 / np.sqrt(deg)).astype(np.float32)
    dinv[N:] = 0.0

    etile = dst // P
    counts = np.bincount(etile, minlength=T)
    order = np.argsort(-counts, kind="stable")
    tile_at = order.reshape(TPC, NCORES)            # [slot, core] -> tile
    tile_core = np.empty(T, np.int64)
    tile_slot = np.empty(T, np.int64)
    tile_core[tile_at.reshape(-1)] = np.tile(np.arange(NCORES), TPC)
    tile_slot[tile_at.reshape(-1)] = np.repeat(np.arange(TPC), NCORES)

    nodes = np.arange(NP, dtype=np.int64)
    qbase, qrem = TPC // NCHUNK, TPC % NCHUNK
    qsizes = [qbase + (1 if j < qrem else 0) for j in range(NCHUNK)]
    qstart = np.cumsum([0] + qsizes)
    qof = np.concatenate([np.full(qsizes[j], j, np.int64)
                          for j in range(NCHUNK)]) if TPC else np.zeros(0, np.int64)
    nslot = tile_slot[nodes // P]
    ncore = tile_core[nodes // P]
    nq = qof[nslot]
    zchunk = nq
    zlocal = (ncore * np.array(qsizes)[nq] * P
              + (nslot - qstart[nq]) * P + nodes % P)

    hrow = []
    colnode = []
    for c in range(NCORES):
        own = list(tile_at[:, c])
        rest = [t for t in range(T) if tile_core[t] != c]
        sq = np.array(own + rest, dtype=np.int64)
        pos = np.empty(T, np.int64)
        pos[sq] = np.arange(T)
        hrow.append(pos[nodes // P] * P + nodes % P)
        colnode.append((sq[:, None] * P + np.arange(P)[None, :]).reshape(-1))

    ecore = tile_core[etile]
    eslot = tile_slot[etile]
    edloc = dst % P
    supers = [list(range(g, min(g + G, TPC))) for g in range(0, TPC, G)]

    def edge_lists(erow_per_core, echunk_per_core):
        lists = [[[None] * NCHUNK for _ in range(TPC)] for _ in range(NCORES)]
        for c in range(NCORES):
            m = ecore == c
            er = erow_per_core[c][m]
            es = eslot[m]
            ed = edloc[m]
            ech = echunk_per_core[c][m]
            key = np.lexsort((ed, ech, es))
            er, es, ed, ech = er[key], es[key], ed[key], ech[key]
            grp = es * NCHUNK + ech
            bounds = np.searchsorted(grp, np.arange(TPC * NCHUNK + 1))
            for s in range(TPC):
                for ch in range(NCHUNK):
                    a, b = bounds[s * NCHUNK + ch], bounds[s * NCHUNK + ch + 1]
                    lists[c][s][ch] = (er[a:b], ed[a:b])
        q16 = np.zeros((TPC, NCHUNK), np.int64)
        for s in range(TPC):
            for ch in range(NCHUNK):
                mx = max(len(lists[c][s][ch][0]) for c in range(NCORES))
                q16[s, ch] = mx
        return lists, q16

    l1, q1 = edge_lists([hrow[c][src] % CH for c in range(NCORES)],
                        [hrow[c][src] // CH for c in range(NCORES)])
    meta1, dl1, ix1 = _build_layer_schedule(l1, q1, supers, TPC)
    l2, q2 = edge_lists([zlocal[src] for _ in range(NCORES)],
                        [zchunk[src] for _ in range(NCORES)])
    meta2, dl2, ix2 = _build_layer_schedule(l2, q2, supers, TPC)

    plan = {
        "N": N, "F_IN": F_IN, "H": H, "C": C, "NP": NP, "T": T, "TPC": TPC,
        "CH": CH, "meta1": meta1, "meta2": meta2, "tile_at": tile_at,
        "qsizes": qsizes, "qstart": list(qstart), "qof": list(qof),
    }
    xpad = np.zeros((NP, F_IN), np.float32)
    xpad[:N] = np.asarray(x, np.float32)
    percore = []
    for c in range(NCORES):
        xT = np.ascontiguousarray(xpad[colnode[c]].T)
        dinvsrc = np.ascontiguousarray(dinv[colnode[c]].reshape(T, P).T)
        ownnodes = (tile_at[:, c][:, None] * P +
                    np.arange(P)[None, :]).reshape(-1)
        dinvdst = np.ascontiguousarray(dinv[ownnodes].reshape(TPC, P).T)
        percore.append({
            "xT": xT,
            "W1": np.asarray(W1, np.float32),
            "W2": np.asarray(W2, np.float32),
            "b1b": np.tile(np.asarray(b1, np.float32)[None, :], (P, 1)),
            "b2b": np.tile(np.asarray(b2, np.float32)[None, :], (P, 1)),
            "iota": np.tile(np.arange(P, dtype=np.float32)[None, :], (P, 1)),
            "dinvsrc": dinvsrc,
            "dinvdst": dinvdst,
            "dl1": dl1[c], "ix1": ix1[c],
            "dl2": dl2[c], "ix2": ix2[c],
        })
    plan["percore"] = percore
    return plan


def _build(plan):
    F_IN, H, C = plan["F_IN"], plan["H"], plan["C"]
    NP, T, TPC, CH = plan["NP"], plan["T"], plan["TPC"], plan["CH"]
    pc0 = plan["percore"][0]
    f32 = mybir.dt.float32

    nc = bacc.Bacc("TRN2", target_bir_lowering=False, debug=False,
                   num_devices=NCORES)
    t_xT = nc.dram_tensor("xT", [F_IN, NP], f32, kind="ExternalInput")
    t_W1 = nc.dram_tensor("W1", [F_IN, H], f32, kind="ExternalInput")
    t_W2 = nc.dram_tensor("W2", [H, C], f32, kind="ExternalInput")
    t_b1b = nc.dram_tensor("b1b", [P, H], f32, kind="ExternalInput")
    t_b2b = nc.dram_tensor("b2b", [P, C], f32, kind="ExternalInput")
    t_iota = nc.dram_tensor("iota", [P, P], f32, kind="ExternalInput")
    t_dsrc = nc.dram_tensor("dinvsrc", [P, T], f32, kind="ExternalInput")
    t_ddst = nc.dram_tensor("dinvdst", [P, TPC], f32, kind="ExternalInput")
    t_dl1 = nc.dram_tensor("dl1", list(pc0["dl1"].shape), f32,
                           kind="ExternalInput")
    t_ix1 = nc.dram_tensor("ix1", list(pc0["ix1"].shape), mybir.dt.int16,
                           kind="ExternalInput")
    t_dl2 = nc.dram_tensor("dl2", list(pc0["dl2"].shape), f32,
                           kind="ExternalInput")
    t_ix2 = nc.dram_tensor("ix2", list(pc0["ix2"].shape), mybir.dt.int16,
                           kind="ExternalInput")
    t_emb = nc.dram_tensor("emb", [TPC * P, H], f32, kind="ExternalOutput")
    t_logp = nc.dram_tensor("logp", [TPC * P, C], f32, kind="ExternalOutput")

    with tile.TileContext(nc) as tc:
        with (
            tc.tile_pool(name="const", bufs=1) as cp,
            tc.tile_pool(name="xin", bufs=2) as xp,
            tc.tile_pool(name="hst", bufs=3) as hp,
            tc.tile_pool(name="gbuf", bufs=2) as gp,
            tc.tile_pool(name="spool", bufs=3) as sp,
            tc.tile_pool(name="epool", bufs=4) as ep,
            tc.tile_pool(name="opool", bufs=3) as op,
            tc.tile_pool(name="psagg", bufs=8, space="PSUM") as pp,
            tc.tile_pool(name="dram", bufs=1, space="DRAM") as dp,
        ):
            def load_const(t, shape, nm, dtype=f32):
                s = cp.tile(shape, dtype, name=nm, tag=nm)
                nc.sync.dma_start(out=s[:], in_=t[:, :])
                return s

            W1s = load_const(t_W1, [F_IN, H], "cW1")
            W2s = load_const(t_W2, [H, C], "cW2")
            b1s = load_const(t_b1b, [P, H], "cb1")
            b2s = load_const(t_b2b, [P, C], "cb2")
            iot = load_const(t_iota, [P, P], "ciota")
            dsrc = load_const(t_dsrc, [P, T], "cdsrc")
            ddst = load_const(t_ddst, [P, TPC], "cddst")
            dl1s = load_const(t_dl1, list(pc0["dl1"].shape), "cdl1")
            ix1s = load_const(t_ix1, list(pc0["ix1"].shape), "cix1", mybir.dt.int16)
            ident = cp.tile([P, P], f32)
            make_identity(nc, ident[:])

            htabs = [dp.tile([CH, H], f32, name=f"htab{i}", tag=f"htab{i}")
                     for i in range(NCHUNK)]
            qsizes, qstart = plan["qsizes"], plan["qstart"]
            qof = plan["qof"]
            zbs = [dp.tile([max(qsizes[j], 1) * P, C], f32, name=f"zb{j}",
                           tag=f"zb{j}") for j in range(NCHUNK)]
            ztabs = [dp.tile([max(qsizes[j], 1) * NCORES * P, C], f32,
                             name=f"ztab{j}", tag=f"ztab{j}")
                     for j in range(NCHUNK)]

            # ---- Phase A: h' table
            XCW = 28 * P if NP % (28 * P) == 0 else P
            ntc = XCW // P
            for x0 in range(0, NP, XCW):
                xc = xp.tile([F_IN, XCW], f32, tag="xc")
                nc.scalar.dma_start(out=xc[:], in_=t_xT[:, x0 : x0 + XCW])
                for jj in range(0, ntc, XB):
                    nbh = min(XB, ntc - jj)
                    hs = hp.tile([P, XB * H], f32, tag="hs")
                    for k in range(nbh):
                        gt = (x0 // P) + jj + k
                        psA = pp.tile([P, H], f32, tag="agg")
                        nc.tensor.matmul(
                            out=psA[:],
                            lhsT=xc[:, (jj + k) * P : (jj + k + 1) * P],
                            rhs=W1s[:],
                            start=True, stop=True,
                        )
                        if gt % 2 == 0:
                            nc.vector.tensor_tensor(
                                out=hs[:, k * H : (k + 1) * H], in0=psA[:],
                                in1=dsrc[:, gt : gt + 1].to_broadcast([P, H]),
                                op=mybir.AluOpType.mult,
                            )
                        else:
                            nc.scalar.activation(
                                out=hs[:, k * H : (k + 1) * H], in_=psA[:],
                                func=mybir.ActivationFunctionType.Copy,
                                scale=dsrc[:, gt : gt + 1],
                            )
                    r0 = (x0 // P + jj) * P
                    chn, rloc = r0 // CH, r0 % CH
                    nc.scalar.dma_start(
                        out=htabs[chn][rloc : rloc + nbh * P, :].rearrange(
                            "(a p) h -> p a h", p=P),
                        in_=hs[:, : nbh * H].rearrange("p (a h) -> p a h", h=H),
                    )

            def emit_call(gb, call, table, ixs, Fdim):
                ch, Q = call["chunk"], call["Q"]
                d0 = call["blk0"] * Fdim
                nc.gpsimd.dma_gather(
                    gb[:, d0 : d0 + (Q // P) * Fdim].rearrange(
                        "p (a h) -> p a h", h=Fdim),
                    table[ch][:, :] if isinstance(table, list)
                    else table[ch * CH : (ch + 1) * CH, :],
                    ixs[:, call["i16"] : call["i16"] + Q // 16],
                    Q, Q, Fdim,
                    single_packet=False,
                )

            def agg_layer(meta, dls, ixs, table, Fdim, epilogue,
                          after_slot=None, window=0):
                sups = meta["supers"]
                wgbufs = {}
                W = min(window, len(sups))
                if W > 1:
                    for g in range(W):
                        if sups[g]["nb"]:
                            wgbufs[g] = gp.tile(
                                [P, sups[g]["nb"] * Fdim], f32, tag="gb",
                                name="gbw", bufs=3)
                    for ci in range(NCHUNK):
                        for g in range(W):
                            for call in sups[g]["calls"]:
                                if call["chunk"] == ci:
                                    emit_call(wgbufs[g], call, table, ixs,
                                              Fdim)
                for gi, sup in enumerate(sups):
                    nb = sup["nb"]
                    psums = {}
                    if nb:
                        if gi in wgbufs:
                            gb = wgbufs[gi]
                        else:
                            gb = gp.tile([P, nb * Fdim], f32, tag="gb",
                                         bufs=3)
                            for call in sup["calls"]:
                                emit_call(gb, call, table, ixs, Fdim)
                        sides = sup["sides"]
                        for i0 in range(0, len(sides), SG):
                            grp = sides[i0 : i0 + SG]
                            k = len(grp)
                            st = sp.tile([P, SG * P], f32, tag="s")
                            c0 = grp[0]["col"]
                            bca = dls[:, c0 : c0 + k].to_broadcast([P, k, P])
                            iap = iot[:, :]
                            iota_b = bass.AP(
                                iap.tensor, iap.offset,
                                [list(iap.ap[0]), [0, k], list(iap.ap[1])],
                            )
                            nc.vector.tensor_tensor(
                                out=st[:, : k * P].rearrange(
                                    "p (a q) -> p a q", q=P),
                                in0=bca, in1=iota_b,
                                op=mybir.AluOpType.is_equal,
                            )
                            for j, sd in enumerate(grp):
                                s = sd["slot"]
                                if s not in psums:
                                    psums[s] = pp.tile([P, Fdim], f32,
                                                       tag="agg", name="aggps")
                                nc.tensor.matmul(
                                    out=psums[s][:],
                                    lhsT=st[:, j * P : (j + 1) * P],
                                    rhs=gb[:, sd["b"] * Fdim :
                                           (sd["b"] + 1) * Fdim],
                                    start=sd["start"], stop=sd["stop"],
                                )
                                if sd["stop"]:
                                    epilogue(sd["slot"], psums[sd["slot"]])
                    for s in sup["slots"]:
                        if s not in psums:
                            psz = pp.tile([P, Fdim], f32, tag="agg")
                            nc.vector.memset(psz[:], 0.0)
                            epilogue(s, psz)
                    if after_slot is not None:
                        for s in sup["slots"]:
                            after_slot(s)

            def epi1(s, ps):
                ownt = op.tile([P, H], f32, tag="own1")
                nc.sync.dma_start(out=ownt[:],
                                  in_=htabs[0][s * P : (s + 1) * P, :])
                t1 = ep.tile([P, H], f32, tag="e1")
                nc.vector.tensor_add(out=t1[:], in0=ps[:], in1=ownt[:])
                t2 = ep.tile([P, H], f32, tag="e2")
                nc.vector.tensor_tensor(
                    out=t2[:], in0=t1[:],
                    in1=ddst[:, s : s + 1].to_broadcast([P, H]),
                    op=mybir.AluOpType.mult,
                )
                t3 = ep.tile([P, H], f32, tag="e3")
                nc.vector.tensor_add(out=t3[:], in0=t2[:], in1=b1s[:])
                h1 = ep.tile([P, H], f32, tag="h1")
                nc.scalar.activation(
                    out=h1[:], in_=t3[:],
                    func=mybir.ActivationFunctionType.Relu,
                )
                nc.sync.dma_start(out=t_emb[s * P : (s + 1) * P, :],
                                  in_=h1[:])
                pt = pp.tile([P, P], f32, tag="agg")
                nc.tensor.transpose(out=pt[:], in_=h1[:], identity=ident[:])
                h1T = ep.tile([P, P], f32, tag="h1T")
                nc.vector.tensor_copy(out=h1T[:], in_=pt[:])
                pz = pp.tile([P, P], f32, tag="agg", name="pzt")
                nc.tensor.matmul(out=pz[:, :C], lhsT=h1T[:], rhs=W2s[:],
                                 start=True, stop=True)
                zt = ep.tile([P, C], f32, tag="zt")
                nc.vector.tensor_tensor(
                    out=zt[:], in0=pz[:, :C],
                    in1=ddst[:, s : s + 1].to_broadcast([P, C]),
                    op=mybir.AluOpType.mult,
                )
                j = qof[s]
                r = (s - qstart[j]) * P
                nc.sync.dma_start(out=zbs[j][r : r + P, :], in_=zt[:])

            def fire_ag(j):
                nc.gpsimd.collective_compute(
                    "AllGather",
                    mybir.AluOpType.bypass,
                    replica_groups=[list(range(NCORES))],
                    ins=[zbs[j].opt()],
                    outs=[ztabs[j].opt()],
                )

            def after_slot1(s):
                for j in range(NCHUNK):
                    if qsizes[j] and s == qstart[j] + qsizes[j] - 1:
                        fire_ag(j)

            agg_layer(plan["meta1"], dl1s, ix1s, htabs, H, epi1,
                      after_slot=after_slot1, window=2)

            dl2s = load_const(t_dl2, list(pc0["dl2"].shape), "cdl2")
            ix2s = load_const(t_ix2, list(pc0["ix2"].shape), "cix2", mybir.dt.int16)

            def epi2(s, ps):
                zown = op.tile([P, C], f32, tag="own2")
                j = qof[s]
                r = (s - qstart[j]) * P
                nc.sync.dma_start(out=zown[:], in_=zbs[j][r : r + P, :])
                t1 = ep.tile([P, C], f32, tag="f1")
                nc.vector.tensor_add(out=t1[:], in0=ps[:], in1=zown[:])
                t2 = ep.tile([P, C], f32, tag="f2")
                nc.vector.tensor_tensor(
                    out=t2[:], in0=t1[:],
                    in1=ddst[:, s : s + 1].to_broadcast([P, C]),
                    op=mybir.AluOpType.mult,
                )
                t3 = ep.tile([P, C], f32, tag="f3")
                nc.vector.tensor_add(out=t3[:], in0=t2[:], in1=b2s[:])
                mx = ep.tile([P, 1], f32, tag="mx")
                nc.vector.reduce_max(out=mx[:], in_=t3[:],
                                     axis=mybir.AxisListType.X)
                xcc = ep.tile([P, C], f32, tag="xcc")
                nc.vector.tensor_tensor(
                    out=xcc[:], in0=t3[:], in1=mx[:].to_broadcast([P, C]),
                    op=mybir.AluOpType.subtract,
                )
                exv = ep.tile([P, C], f32, tag="exv")
                smv = ep.tile([P, 1], f32, tag="smv")
                nc.scalar.activation(
                    out=exv[:], in_=xcc[:],
                    func=mybir.ActivationFunctionType.Exp,
                    accum_out=smv[:],
                )
                lsv = ep.tile([P, 1], f32, tag="lsv")
                nc.scalar.activation(
                    out=lsv[:], in_=smv[:],
                    func=mybir.ActivationFunctionType.Ln,
                )
                ov = ep.tile([P, C], f32, tag="ov")
                nc.vector.tensor_tensor(
                    out=ov[:], in0=xcc[:], in1=lsv[:].to_broadcast([P, C]),
                    op=mybir.AluOpType.subtract,
                )
                nc.sync.dma_start(out=t_logp[s * P : (s + 1) * P, :],
                                  in_=ov[:])

            agg_layer(plan["meta2"], dl2s, ix2s, ztabs, C, epi2)

    nc.compile()
    return nc


def _in_maps(plan):
    maps = []
    for c in range(NCORES):
        pc = plan["percore"][c]
        maps.append({
            "xT": pc["xT"], "W1": pc["W1"], "W2": pc["W2"],
            "b1b": pc["b1b"], "b2b": pc["b2b"], "iota": pc["iota"],
            "dinvsrc": pc["dinvsrc"], "dinvdst": pc["dinvdst"],
            "dl1": pc["dl1"], "ix1": pc["ix1"],
            "dl2": pc["dl2"], "ix2": pc["ix2"],
        })
    return maps


def _assemble(plan, results):
    N, H, C, TPC = plan["N"], plan["H"], plan["C"], plan["TPC"]
    NP = plan["NP"]
    tile_at = plan["tile_at"]
    emb = np.zeros((NP, H), np.float32)
    logp = np.zeros((NP, C), np.float32)
    for c in range(NCORES):
        e = results[c]["emb"]
        l = results[c]["logp"]
        for s in range(TPC):
            gt = tile_at[s, c]
            emb[gt * P : (gt + 1) * P] = e[s * P : (s + 1) * P]
            logp[gt * P : (gt + 1) * P] = l[s * P : (s + 1) * P]
    return logp[:N], emb[:N]


def kernel(x, W1, b1, W2, b2, edge_index, _trace=False, _want_time=False):
    plan = _plan(np.asarray(x), np.asarray(W1), np.asarray(b1),
                 np.asarray(W2), np.asarray(b2), np.asarray(edge_index))
    nc = _build(plan)
    res = run_bass_kernel_spmd(nc, _in_maps(plan),
                               core_ids=list(range(NCORES)), trace=_trace)
    logp, emb = _assemble(plan, res.results)
    if _want_time:
        return (logp, emb), res.exec_time_ns
    return logp, emb


# revision 13
# speedup vs baseline: 1.1344x; 1.0201x over previous
"""GCN (2-layer, symmetric-normalized adjacency) on 8 TRN2 NeuronCores.

Strategy:
  - Full f32. Host does graph preprocessing only (normalization constants,
    partitioning, index tables); all FLOPs on x/W run on device.
  - Nodes padded to NP; dst tiles of 128 nodes; tiles assigned to cores with
    count balancing (sort by edge count, deal round-robin).
  - GEMM1 per-core computes h' = dinv[src]*(x@W1) for ALL nodes into a DRAM
    table (per-core column order: own tiles first, so self-loop rows sit at
    static offsets).
  - Edge messages gathered from the table with bulk dma_gather (int16
    indices -> 4 src-row chunks; per-(tile,chunk) quotas padded to the max
    across cores so the program is SPMD-uniform).
  - Aggregation = one-hot matmuls on the TensorEngine: S[p,j] =
    (dstloc[p]==j), PSUM-accumulated per dst tile; self-loop row added from
    the table; epilogue relu(dinv*(agg+own)+b1).
  - z' = dinv*(h1@W2) shards AllGathered into a z table; layer 2 repeats the
    gather/aggregate structure and ends in log_softmax.
"""

import sys
import types

import numpy as np


def _install_ntff_hook():
    if "antenv.axon_hooks" in sys.modules:
        return
    try:
        from trn_agent_boot.trn_boot import _ntff_profile_via_ctypes

        hook = _ntff_profile_via_ctypes("/opt/axon/libaxon_pjrt.so")
    except Exception:
        hook = None
    mod = types.ModuleType("antenv.axon_hooks")
    mod.get_axon_ntff_profile_hook = lambda: hook
    mod.set_axon_ntff_profile_hook = lambda h: None
    sys.modules["antenv.axon_hooks"] = mod


_install_ntff_hook()

import concourse.bass as bass
import concourse.bacc as bacc
import concourse.tile as tile
import concourse.mybir as mybir
from concourse.bass_utils import run_bass_kernel_spmd
from concourse.masks import make_identity

P = 128
NCORES = 8
NCHUNK = 4
G = 6        # dst tiles per superbatch
SG = 16      # one-hot columns per S tile
XB = 4       # GEMM1 tiles per staging/DMA batch
INVALID = 1000.0


def _round_up(x, m):
    return -(-x // m) * m


def _build_layer_schedule(lists, q16, supers, TPC):
    """Static (core-uniform) schedule for one layer + per-core dstloc/idx.

    lists[c][s][ch] = (rows_in_chunk, dst_local) per core/slot/chunk,
    q16[s, ch] = padded (max-over-cores) count, multiple of 16.
    """
    meta = {"supers": []}
    dl_cols = []     # column fill instructions: (col, seg info)
    col_count = 0
    idx16_count = 0
    idxw = [[] for _ in range(NCORES)]

    for slots in supers:
        calls = []
        segs = []    # (slot, start_pos, qlen, chunk)
        pos = 0
        for ch in range(NCHUNK):
            q_sum = int(sum(q16[s, ch] for s in slots))
            if q_sum == 0:
                continue
            Q = _round_up(q_sum, P)
            calls.append({"chunk": ch, "Q": Q, "i16": idx16_count,
                          "blk0": pos // P})
            for c in range(NCORES):
                arr = np.zeros(Q, np.int64)
                o = 0
                for s in slots:
                    q = int(q16[s, ch])
                    if q == 0:
                        continue
                    rows = lists[c][s][ch][0]
                    arr[o : o + len(rows)] = rows
                    o += q
                w = arr.astype(np.int16).reshape(Q // 16, 16).T
                idxw[c].append(np.tile(w, (NCORES, 1)))
            for s in slots:
                q = int(q16[s, ch])
                if q:
                    segs.append((s, pos, q, ch))
                    pos += q
            pos += Q - q_sum
            idx16_count += Q // 16
        M = pos
        assert M % P == 0
        nb = M // P
        sides = []
        slot_sides = {}
        for b in range(nb):
            lo, hi = b * P, (b + 1) * P
            for (s, sp, ln, ch) in segs:
                if sp < hi and sp + ln > lo:
                    sd = {"b": b, "col": col_count, "slot": s,
                          "sp": sp, "ln": ln, "chunk": ch}
                    sides.append(sd)
                    slot_sides.setdefault(s, []).append(sd)
                    col_count += 1
        for s, sl in slot_sides.items():
            for sd in sl:
                sd["start"] = sd is sl[0]
                sd["stop"] = sd is sl[-1]
        meta["supers"].append({"slots": slots, "calls": calls, "nb": nb,
                               "sides": sides})
    meta["ncols"] = max(col_count, 1)
    meta["nidx16"] = max(idx16_count, 1)

    dl = [np.full((P, meta["ncols"]), INVALID, np.float32)
          for _ in range(NCORES)]
    for sup in meta["supers"]:
        for sd in sup["sides"]:
            sp, ln, s, ch = sd["sp"], sd["ln"], sd["slot"], sd["chunk"]
            lo, hi = sd["b"] * P, (sd["b"] + 1) * P
            a = max(sp, lo)
            b_ = min(sp + ln, hi)
            r0 = a - sp
            for c in range(NCORES):
                ed = lists[c][s][ch][1]
                r1 = min(b_ - sp, len(ed))
                if r1 > r0:
                    pidx = (a - lo) + np.arange(r1 - r0)
                    dl[c][pidx, sd["col"]] = ed[r0:r1]

    idxcat = []
    for c in range(NCORES):
        if idxw[c]:
            idxcat.append(np.ascontiguousarray(
                np.concatenate(idxw[c], axis=1)))
        else:
            idxcat.append(np.zeros((P, 1), np.int16))
    return meta, dl, idxcat


def _plan(x, W1, b1, W2, b2, edge_index):
    N, F_IN = x.shape
    H = W1.shape[1]
    C = W2.shape[1]
    src = np.asarray(edge_index[0], dtype=np.int64)
    dst = np.asarray(edge_index[1], dtype=np.int64)

    NP = _round_up(N, P * NCORES)
    T = NP // P
    TPC = T // NCORES
    CH = NP // NCHUNK
    assert CH <= 32767 and CH % P == 0

    deg = np.bincount(dst, minlength=NP).astype(np.float64) + 1.0
    dinv = (1.0 / np.sqrt(deg)).astype(np.float32)
    dinv[N:] = 0.0

    etile = dst // P
    counts = np.bincount(etile, minlength=T)
    order = np.argsort(-counts, kind="stable")
    tile_at = order.reshape(TPC, NCORES)            # [slot, core] -> tile
    tile_core = np.empty(T, np.int64)
    tile_slot = np.empty(T, np.int64)
    tile_core[tile_at.reshape(-1)] = np.tile(np.arange(NCORES), TPC)
    tile_slot[tile_at.reshape(-1)] = np.repeat(np.arange(TPC), NCORES)

    nodes = np.arange(NP, dtype=np.int64)
    qbase, qrem = TPC // NCHUNK, TPC % NCHUNK
    qsizes = [qbase + (1 if j < qrem else 0) for j in range(NCHUNK)]
    qstart = np.cumsum([0] + qsizes)
    qof = np.concatenate([np.full(qsizes[j], j, np.int64)
                          for j in range(NCHUNK)]) if TPC else np.zeros(0, np.int64)
    nslot = tile_slot[nodes // P]
    ncore = tile_core[nodes // P]
    nq = qof[nslot]
    zchunk = nq
    zlocal = (ncore * np.array(qsizes)[nq] * P
              + (nslot - qstart[nq]) * P + nodes % P)

    hrow = []
    colnode = []
    for c in range(NCORES):
        own = list(tile_at[:, c])
        rest = [t for t in range(T) if tile_core[t] != c]
        sq = np.array(own + rest, dtype=np.int64)
        pos = np.empty(T, np.int64)
        pos[sq] = np.arange(T)
        hrow.append(pos[nodes // P] * P + nodes % P)
        colnode.append((sq[:, None] * P + np.arange(P)[None, :]).reshape(-1))

    ecore = tile_core[etile]
    eslot = tile_slot[etile]
    edloc = dst % P
    supers = [list(range(g, min(g + G, TPC))) for g in range(0, TPC, G)]

    def edge_lists(erow_per_core, echunk_per_core):
        lists = [[[None] * NCHUNK for _ in range(TPC)] for _ in range(NCORES)]
        for c in range(NCORES):
            m = ecore == c
            er = erow_per_core[c][m]
            es = eslot[m]
            ed = edloc[m]
            ech = echunk_per_core[c][m]
            key = np.lexsort((ed, ech, es))
            er, es, ed, ech = er[key], es[key], ed[key], ech[key]
            grp = es * NCHUNK + ech
            bounds = np.searchsorted(grp, np.arange(TPC * NCHUNK + 1))
            for s in range(TPC):
                for ch in range(NCHUNK):
                    a, b = bounds[s * NCHUNK + ch], bounds[s * NCHUNK + ch + 1]
                    lists[c][s][ch] = (er[a:b], ed[a:b])
        q16 = np.zeros((TPC, NCHUNK), np.int64)
        for s in range(TPC):
            for ch in range(NCHUNK):
                mx = max(len(lists[c][s][ch][0]) for c in range(NCORES))
                q16[s, ch] = mx
        return lists, q16

    l1, q1 = edge_lists([hrow[c][src] % CH for c in range(NCORES)],
                        [hrow[c][src] // CH for c in range(NCORES)])
    meta1, dl1, ix1 = _build_layer_schedule(l1, q1, supers, TPC)
    l2, q2 = edge_lists([zlocal[src] for _ in range(NCORES)],
                        [zchunk[src] for _ in range(NCORES)])
    meta2, dl2, ix2 = _build_layer_schedule(l2, q2, supers, TPC)

    plan = {
        "N": N, "F_IN": F_IN, "H": H, "C": C, "NP": NP, "T": T, "TPC": TPC,
        "CH": CH, "meta1": meta1, "meta2": meta2, "tile_at": tile_at,
        "qsizes": qsizes, "qstart": list(qstart), "qof": list(qof),
    }
    xpad = np.zeros((NP, F_IN), np.float32)
    xpad[:N] = np.asarray(x, np.float32)
    percore = []
    for c in range(NCORES):
        xT = np.ascontiguousarray(xpad[colnode[c]].T)
        dinvsrc = np.ascontiguousarray(dinv[colnode[c]].reshape(T, P).T)
        ownnodes = (tile_at[:, c][:, None] * P +
                    np.arange(P)[None, :]).reshape(-1)
        dinvdst = np.ascontiguousarray(dinv[ownnodes].reshape(TPC, P).T)
        percore.append({
            "xT": xT,
            "W1": np.asarray(W1, np.float32),
            "W2": np.asarray(W2, np.float32),
            "b1b": np.tile(np.asarray(b1, np.float32)[None, :], (P, 1)),
            "b2b": np.tile(np.asarray(b2, np.float32)[None, :], (P, 1)),
            "iota": np.tile(np.arange(P, dtype=np.float32)[None, :], (P, 1)),
            "dinvsrc": dinvsrc,
            "dinvdst": dinvdst,
            "dl1": dl1[c], "ix1": ix1[c],
            "dl2": dl2[c], "ix2": ix2[c],
        })
    plan["percore"] = percore
    return plan


def _build(plan):
    F_IN, H, C = plan["F_IN"], plan["H"], plan["C"]
    NP, T, TPC, CH = plan["NP"], plan["T"], plan["TPC"], plan["CH"]
    pc0 = plan["percore"][0]
    f32 = mybir.dt.float32

    nc = bacc.Bacc("TRN2", target_bir_lowering=False, debug=False,
                   num_devices=NCORES)
    t_xT = nc.dram_tensor("xT", [F_IN, NP], f32, kind="ExternalInput")
    t_W1 = nc.dram_tensor("W1", [F_IN, H], f32, kind="ExternalInput")
    t_W2 = nc.dram_tensor("W2", [H, C], f32, kind="ExternalInput")
    t_b1b = nc.dram_tensor("b1b", [P, H], f32, kind="ExternalInput")
    t_b2b = nc.dram_tensor("b2b", [P, C], f32, kind="ExternalInput")
    t_iota = nc.dram_tensor("iota", [P, P], f32, kind="ExternalInput")
    t_dsrc = nc.dram_tensor("dinvsrc", [P, T], f32, kind="ExternalInput")
    t_ddst = nc.dram_tensor("dinvdst", [P, TPC], f32, kind="ExternalInput")
    t_dl1 = nc.dram_tensor("dl1", list(pc0["dl1"].shape), f32,
                           kind="ExternalInput")
    t_ix1 = nc.dram_tensor("ix1", list(pc0["ix1"].shape), mybir.dt.int16,
                           kind="ExternalInput")
    t_dl2 = nc.dram_tensor("dl2", list(pc0["dl2"].shape), f32,
                           kind="ExternalInput")
    t_ix2 = nc.dram_tensor("ix2", list(pc0["ix2"].shape), mybir.dt.int16,
                           kind="ExternalInput")
    t_emb = nc.dram_tensor("emb", [TPC * P, H], f32, kind="ExternalOutput")
    t_logp = nc.dram_tensor("logp", [TPC * P, C], f32, kind="ExternalOutput")

    with tile.TileContext(nc) as tc:
        with (
            tc.tile_pool(name="const", bufs=1) as cp,
            tc.tile_pool(name="xin", bufs=2) as xp,
            tc.tile_pool(name="hst", bufs=3) as hp,
            tc.tile_pool(name="gbuf", bufs=2) as gp,
            tc.tile_pool(name="spool", bufs=3) as sp,
            tc.tile_pool(name="epool", bufs=4) as ep,
            tc.tile_pool(name="opool", bufs=3) as op,
            tc.tile_pool(name="psagg", bufs=8, space="PSUM") as pp,
            tc.tile_pool(name="dram", bufs=1, space="DRAM") as dp,
        ):
            def load_const(t, shape, nm, dtype=f32):
                s = cp.tile(shape, dtype, name=nm, tag=nm)
                nc.sync.dma_start(out=s[:], in_=t[:, :])
                return s

            W1s = load_const(t_W1, [F_IN, H], "cW1")
            W2s = load_const(t_W2, [H, C], "cW2")
            b1s = load_const(t_b1b, [P, H], "cb1")
            b2s = load_const(t_b2b, [P, C], "cb2")
            iot = load_const(t_iota, [P, P], "ciota")
            dsrc = load_const(t_dsrc, [P, T], "cdsrc")
            ddst = load_const(t_ddst, [P, TPC], "cddst")
            dl1s = load_const(t_dl1, list(pc0["dl1"].shape), "cdl1")
            ix1s = load_const(t_ix1, list(pc0["ix1"].shape), "cix1", mybir.dt.int16)
            ident = cp.tile([P, P], f32)
            make_identity(nc, ident[:])

            htabs = [dp.tile([CH, H], f32, name=f"htab{i}", tag=f"htab{i}")
                     for i in range(NCHUNK)]
            qsizes, qstart = plan["qsizes"], plan["qstart"]
            qof = plan["qof"]
            zbs = [dp.tile([max(qsizes[j], 1) * P, C], f32, name=f"zb{j}",
                           tag=f"zb{j}") for j in range(NCHUNK)]
            ztabs = [dp.tile([max(qsizes[j], 1) * NCORES * P, C], f32,
                             name=f"ztab{j}", tag=f"ztab{j}")
                     for j in range(NCHUNK)]

            # ---- Phase A: h' table
            XCW = 28 * P if NP % (28 * P) == 0 else P
            ntc = XCW // P
            for x0 in range(0, NP, XCW):
                xc = xp.tile([F_IN, XCW], f32, tag="xc")
                nc.scalar.dma_start(out=xc[:], in_=t_xT[:, x0 : x0 + XCW])
                for jj in range(0, ntc, XB):
                    nbh = min(XB, ntc - jj)
                    gt0 = (x0 // P) + jj
                    hs = hp.tile([P, XB * H], f32, tag="hs")
                    psB = pp.tile([P, XB * H], f32, tag="agg", name="psB")
                    for k in range(nbh):
                        nc.tensor.matmul(
                            out=psB[:, k * H : (k + 1) * H],
                            lhsT=xc[:, (jj + k) * P : (jj + k + 1) * P],
                            rhs=W1s[:],
                            start=True, stop=True,
                        )
                    nc.vector.tensor_tensor(
                        out=hs[:, : nbh * H].rearrange("p (a h) -> p a h", h=H),
                        in0=psB[:, : nbh * H].rearrange("p (a h) -> p a h", h=H),
                        in1=dsrc[:, gt0 : gt0 + nbh].to_broadcast([P, nbh, H]),
                        op=mybir.AluOpType.mult,
                    )
                    r0 = (x0 // P + jj) * P
                    chn, rloc = r0 // CH, r0 % CH
                    nc.scalar.dma_start(
                        out=htabs[chn][rloc : rloc + nbh * P, :].rearrange(
                            "(a p) h -> p a h", p=P),
                        in_=hs[:, : nbh * H].rearrange("p (a h) -> p a h", h=H),
                    )

            def emit_call(gb, call, table, ixs, Fdim):
                ch, Q = call["chunk"], call["Q"]
                d0 = call["blk0"] * Fdim
                nc.gpsimd.dma_gather(
                    gb[:, d0 : d0 + (Q // P) * Fdim].rearrange(
                        "p (a h) -> p a h", h=Fdim),
                    table[ch][:, :] if isinstance(table, list)
                    else table[ch * CH : (ch + 1) * CH, :],
                    ixs[:, call["i16"] : call["i16"] + Q // 16],
                    Q, Q, Fdim,
                    single_packet=False,
                )

            def agg_layer(meta, dls, ixs, table, Fdim, epilogue,
                          after_slot=None, window=0):
                sups = meta["supers"]
                wgbufs = {}
                W = min(window, len(sups))
                if W > 1:
                    for g in range(W):
                        if sups[g]["nb"]:
                            wgbufs[g] = gp.tile(
                                [P, sups[g]["nb"] * Fdim], f32, tag="gb",
                                name="gbw", bufs=3)
                    for ci in range(NCHUNK):
                        for g in range(W):
                            for call in sups[g]["calls"]:
                                if call["chunk"] == ci:
                                    emit_call(wgbufs[g], call, table, ixs,
                                              Fdim)
                for gi, sup in enumerate(sups):
                    nb = sup["nb"]
                    psums = {}
                    if nb:
                        if gi in wgbufs:
                            gb = wgbufs[gi]
                        else:
                            gb = gp.tile([P, nb * Fdim], f32, tag="gb",
                                         bufs=3)
                            for call in sup["calls"]:
                                emit_call(gb, call, table, ixs, Fdim)
                        sides = sup["sides"]
                        for i0 in range(0, len(sides), SG):
                            grp = sides[i0 : i0 + SG]
                            k = len(grp)
                            st = sp.tile([P, SG * P], f32, tag="s")
                            c0 = grp[0]["col"]
                            bca = dls[:, c0 : c0 + k].to_broadcast([P, k, P])
                            iap = iot[:, :]
                            iota_b = bass.AP(
                                iap.tensor, iap.offset,
                                [list(iap.ap[0]), [0, k], list(iap.ap[1])],
                            )
                            nc.vector.tensor_tensor(
                                out=st[:, : k * P].rearrange(
                                    "p (a q) -> p a q", q=P),
                                in0=bca, in1=iota_b,
                                op=mybir.AluOpType.is_equal,
                            )
                            for j, sd in enumerate(grp):
                                s = sd["slot"]
                                if s not in psums:
                                    psums[s] = pp.tile([P, Fdim], f32,
                                                       tag="agg", name="aggps")
                                nc.tensor.matmul(
                                    out=psums[s][:],
                                    lhsT=st[:, j * P : (j + 1) * P],
                                    rhs=gb[:, sd["b"] * Fdim :
                                           (sd["b"] + 1) * Fdim],
                                    start=sd["start"], stop=sd["stop"],
                                )
                                if sd["stop"]:
                                    epilogue(sd["slot"], psums[sd["slot"]])
                    for s in sup["slots"]:
                        if s not in psums:
                            psz = pp.tile([P, Fdim], f32, tag="agg")
                            nc.vector.memset(psz[:], 0.0)
                            epilogue(s, psz)
                    if after_slot is not None:
                        for s in sup["slots"]:
                            after_slot(s)

            def epi1(s, ps):
                ownt = op.tile([P, H], f32, tag="own1")
                nc.sync.dma_start(out=ownt[:],
                                  in_=htabs[0][s * P : (s + 1) * P, :])
                t1 = ep.tile([P, H], f32, tag="e1")
                nc.vector.tensor_add(out=t1[:], in0=ps[:], in1=ownt[:])
                t2 = ep.tile([P, H], f32, tag="e2")
                nc.vector.tensor_tensor(
                    out=t2[:], in0=t1[:],
                    in1=ddst[:, s : s + 1].to_broadcast([P, H]),
                    op=mybir.AluOpType.mult,
                )
                t3 = ep.tile([P, H], f32, tag="e3")
                nc.vector.tensor_add(out=t3[:], in0=t2[:], in1=b1s[:])
                h1 = ep.tile([P, H], f32, tag="h1")
                nc.scalar.activation(
                    out=h1[:], in_=t3[:],
                    func=mybir.ActivationFunctionType.Relu,
                )
                nc.sync.dma_start(out=t_emb[s * P : (s + 1) * P, :],
                                  in_=h1[:])
                pt = pp.tile([P, P], f32, tag="agg")
                nc.tensor.transpose(out=pt[:], in_=h1[:], identity=ident[:])
                h1T = ep.tile([P, P], f32, tag="h1T")
                nc.vector.tensor_copy(out=h1T[:], in_=pt[:])
                pz = pp.tile([P, P], f32, tag="agg", name="pzt")
                nc.tensor.matmul(out=pz[:, :C], lhsT=h1T[:], rhs=W2s[:],
                                 start=True, stop=True)
                zt = ep.tile([P, C], f32, tag="zt")
                nc.vector.tensor_tensor(
                    out=zt[:], in0=pz[:, :C],
                    in1=ddst[:, s : s + 1].to_broadcast([P, C]),
                    op=mybir.AluOpType.mult,
                )
                j = qof[s]
                r = (s - qstart[j]) * P
                nc.sync.dma_start(out=zbs[j][r : r + P, :], in_=zt[:])

            def fire_ag(j):
                nc.gpsimd.collective_compute(
                    "AllGather",
                    mybir.AluOpType.bypass,
                    replica_groups=[list(range(NCORES))],
                    ins=[zbs[j].opt()],
                    outs=[ztabs[j].opt()],
                )

            def after_slot1(s):
                for j in range(NCHUNK):
                    if qsizes[j] and s == qstart[j] + qsizes[j] - 1:
                        fire_ag(j)

            agg_layer(plan["meta1"], dl1s, ix1s, htabs, H, epi1,
                      after_slot=after_slot1, window=2)

            dl2s = load_const(t_dl2, list(pc0["dl2"].shape), "cdl2")
            ix2s = load_const(t_ix2, list(pc0["ix2"].shape), "cix2", mybir.dt.int16)

            def epi2(s, ps):
                zown = op.tile([P, C], f32, tag="own2")
                j = qof[s]
                r = (s - qstart[j]) * P
                nc.sync.dma_start(out=zown[:], in_=zbs[j][r : r + P, :])
                t1 = ep.tile([P, C], f32, tag="f1")
                nc.vector.tensor_add(out=t1[:], in0=ps[:], in1=zown[:])
                t2 = ep.tile([P, C], f32, tag="f2")
                nc.vector.tensor_tensor(
                    out=t2[:], in0=t1[:],
                    in1=ddst[:, s : s + 1].to_broadcast([P, C]),
                    op=mybir.AluOpType.mult,
                )
                t3 = ep.tile([P, C], f32, tag="f3")
                nc.vector.tensor_add(out=t3[:], in0=t2[:], in1=b2s[:])
                mx = ep.tile([P, 1], f32, tag="mx")
                nc.vector.reduce_max(out=mx[:], in_=t3[:],
                                     axis=mybir.AxisListType.X)
                xcc = ep.tile([P, C], f32, tag="xcc")
                nc.vector.tensor_tensor(
                    out=xcc[:], in0=t3[:], in1=mx[:].to_broadcast([P, C]),
                    op=mybir.AluOpType.subtract,
                )
                exv = ep.tile([P, C], f32, tag="exv")
                smv = ep.tile([P, 1], f32, tag="smv")
                nc.scalar.activation(
                    out=exv[:], in_=xcc[:],
                    func=mybir.ActivationFunctionType.Exp,
                    accum_out=smv[:],
                )
                lsv = ep.tile([P, 1], f32, tag="lsv")
                nc.scalar.activation(
                    out=lsv[:], in_=smv[:],
                    func=mybir.ActivationFunctionType.Ln,
                )
                ov = ep.tile([P, C], f32, tag="ov")
                nc.vector.tensor_tensor(
                    out=ov[:], in0=xcc[:], in1=lsv[:].to_broadcast([P, C]),
                    op=mybir.AluOpType.subtract,
                )
                nc.sync.dma_start(out=t_logp[s * P : (s + 1) * P, :],
                                  in_=ov[:])

            agg_layer(plan["meta2"], dl2s, ix2s, ztabs, C, epi2)

    nc.compile()
    return nc


def _in_maps(plan):
    maps = []
    for c in range(NCORES):
        pc = plan["percore"][c]
        maps.append({
            "xT": pc["xT"], "W1": pc["W1"], "W2": pc["W2"],
            "b1b": pc["b1b"], "b2b": pc["b2b"], "iota": pc["iota"],
            "dinvsrc": pc["dinvsrc"], "dinvdst": pc["dinvdst"],
            "dl1": pc["dl1"], "ix1": pc["ix1"],
            "dl2": pc["dl2"], "ix2": pc["ix2"],
        })
    return maps


def _assemble(plan, results):
    N, H, C, TPC = plan["N"], plan["H"], plan["C"], plan["TPC"]
    NP = plan["NP"]
    tile_at = plan["tile_at"]
    emb = np.zeros((NP, H), np.float32)
    logp = np.zeros((NP, C), np.float32)
    for c in range(NCORES):
        e = results[c]["emb"]
        l = results[c]["logp"]
        for s in range(TPC):
            gt = tile_at[s, c]
            emb[gt * P : (gt + 1) * P] = e[s * P : (s + 1) * P]
            logp[gt * P : (gt + 1) * P] = l[s * P : (s + 1) * P]
    return logp[:N], emb[:N]


def kernel(x, W1, b1, W2, b2, edge_index, _trace=False, _want_time=False):
    plan = _plan(np.asarray(x), np.asarray(W1), np.asarray(b1),
                 np.asarray(W2), np.asarray(b2), np.asarray(edge_index))
    nc = _build(plan)
    res = run_bass_kernel_spmd(nc, _in_maps(plan),
                               core_ids=list(range(NCORES)), trace=_trace)
    logp, emb = _assemble(plan, res.results)
    if _want_time:
        return (logp, emb), res.exec_time_ns
    return logp, emb
